# revision 1
# baseline (speedup 1.0000x reference)
"""Trainium2 Bass kernel for nn_MultiHeadAttention_88210038326473.

Reference computation (B=4, S=2048, HID=2048, H=16, DH=128):
    Q = queries @ Wq.T + bq ; K = keys @ Wk.T + bk ; V = keys @ Wv.T + bv
    per-head scores = Qh Kh^T / sqrt(HID), key-padding + causal mask,
    softmax, out = attn @ Vh, concat heads, + queries residual.

Sharding: 8 cores = 4 batches x 2 head-groups (8 heads each). Each core
computes out[b, :, hg*1024:(hg+1)*1024] (stored transposed [1024, 2048];
host transposes back and assembles).

Device algorithm per core:
  Phase KV: KT = (keys @ Wk.T).T  [1024e, 2048s] and V = keys @ Wv.T
            [2048s, 1024e] -> DRAM scratch (fp32r).
  Phase Q:  QT = (queries @ Wq.T).T [1024e, 2048s] -> resident SBUF.
  Attention per (head, q-chunk of 512): scores computed transposed
            sT[k,q] = KT_h^T QT_h per 128-k-tile (causal tiles only; the
            diagonal-band tiles only compute the valid right part),
            expT = Exp(scale*sT + key_pad_bias) (no max subtraction --
            scores are O(1) so exp never overflows; masked -> exp==0),
            diagonal 128x128 blocks masked by a 0/1 triangle, V-matmul
            accumulates outT[d,q] += V_tile^T expT, row-sums accumulated
            in PSUM via a ones-column matmul, reciprocal, broadcast back
            via a K=1 ones-matmul, normalize + residual.

All matmuls use float32r (~13-bit mantissa, full PE rate at N>=256).
"""

import math

import numpy as np

B, S, HID, H, DH = 4, 2048, 2048, 16, 128
NCORES = 8
HPC = 8          # heads per core
EH = HPC * DH    # 1024 e-dims per core
SCALE = 1.0 / math.sqrt(HID)
SC = 256         # projection s-chunk
NSC = S // SC    # 8
QC = 512         # attention q-chunk
NQC = S // QC    # 4
NKT = S // DH    # 16 k-tiles
NF = HID // DH   # 16 f-tiles (contraction)
PC = 512         # projection s-chunk (matmul moving N)
NPC = S // PC    # 4
NEG_BIAS = np.float32(-1.0e30)
ACT_DT = "bf16"   # "bf16" or "f32r": matmul operand precision
COMPUTE_MAX_WAITS = 1  # waits allowed on non-CTRL instructions before splitting


CTRL_OPS = ("InstDrain", "InstNoOp", "InstEventSemaphore", "InstISA")


def _split_excess_waits(nc, max_waits=1, compute_max_waits=None):
    """walrus in this container rejects >1 sem-wait per CTRL-class instruction.
    Move excess waits onto preceding NoOps on the same engine. Compute-class
    instructions may support more waits (compute_max_waits)."""
    import concourse.mybir as mybir

    if compute_max_waits is None:
        compute_max_waits = max_waits
    n_split = 0
    for fn in nc.m.functions:
        for blk in fn.blocks:
            insts = list(blk.instructions)
            out = []
            changed = False
            for ins in insts:
                lim = (
                    max_waits
                    if type(ins).__name__ in CTRL_OPS
                    else compute_max_waits
                )
                si = ins.sync_info
                if si is not None and si.on_wait and len(si.on_wait) > lim:
                    waits = list(si.on_wait)
                    carriers, rest = waits[:-lim], waits[-lim:]
                    for i in range(0, len(carriers), max_waits):
                        chunk = carriers[i : i + max_waits]
                        out.append(
                            mybir.InstNoOp(
                                name=f"{ins.name}-ws{i}",
                                engine=ins.engine,
                                bass_nofuse=True,
                                sync_info=mybir.SyncInfo(on_wait=chunk, on_update=[]),
                            )
                        )
                        n_split += 1
                    ins.sync_info = mybir.SyncInfo(
                        on_wait=rest, on_update=list(si.on_update)
                    )
                    changed = True
                out.append(ins)
            if changed:
                blk.instructions = out
    return n_split


_CACHE = {}


def _build(fast=True, phases=("k", "v", "q", "attn"), reps=1, act_dt="bf16",
           scale=None):
    """Build the (core-uniform) Bass program. Returns nc.

    fast=True drops the key-padding bias from the exp (valid when no key is
    padding, i.e. kmask all-ones -- checked on host). fast=False applies the
    per-k-tile padding bias (general path). reps>1 repeats the whole body
    (timing instrument: slope isolates NEFF time from launch overhead).
    act_dt: "bf16" or "f32r" -- dtype of weights/activations/probabilities
    fed to the matmuls (PSUM accumulation and the residual path are always
    fp32)."""
    scale = scale or {}
    key = ("nc", fast, tuple(phases), reps, act_dt, tuple(sorted(scale.items())))
    if key in _CACHE:
        return _CACHE[key]

    import concourse.bass as bass
    import concourse.mybir as mybir
    from concourse.tile import TileContext

    F32 = mybir.dt.float32
    F32R = mybir.dt.float32r
    ADT = mybir.dt.bfloat16 if act_dt == "bf16" else F32R
    EXP = mybir.ActivationFunctionType.Exp
    IDENT = mybir.ActivationFunctionType.Identity

    nc = bass.Bass("TRN2", target_bir_lowering=False, debug=False)

    qT = nc.dram_tensor("qT", [HID, S], ADT, kind="ExternalInput")
    kT = nc.dram_tensor("kT", [HID, S], ADT, kind="ExternalInput")
    wqT = nc.dram_tensor("wqT", [HID, EH], ADT, kind="ExternalInput")
    wkT = nc.dram_tensor("wkT", [HID, EH], ADT, kind="ExternalInput")
    wvT = nc.dram_tensor("wvT", [HID, EH], ADT, kind="ExternalInput")
    bq_d = nc.dram_tensor("bq_d", [DH, HPC], F32, kind="ExternalInput")
    bk_d = nc.dram_tensor("bk_d", [DH, HPC], F32, kind="ExternalInput")
    bv_d = nc.dram_tensor("bv_d", [1, EH], ADT, kind="ExternalInput")
    kbias_d = nc.dram_tensor("kbias_d", [DH, NKT], F32, kind="ExternalInput")
    tri_d = nc.dram_tensor("tri_d", [DH, DH], ADT, kind="ExternalInput")
    ones_c_d = nc.dram_tensor("ones_c_d", [DH, 1], ADT, kind="ExternalInput")
    ones_r_d = nc.dram_tensor("ones_r_d", [1, DH], F32R, kind="ExternalInput")
    ones_ra_d = nc.dram_tensor("ones_ra_d", [1, DH], ADT, kind="ExternalInput")
    resid_d = nc.dram_tensor("resid_d", [EH, S], F32, kind="ExternalInput")
    outT_d = nc.dram_tensor("outT_d", [EH, S], F32, kind="ExternalOutput")

    # 3D views with the 128-partition dim innermost on rows
    qT3 = qT[:].rearrange("(f p) s -> p f s", p=DH)
    kT3 = kT[:].rearrange("(f p) s -> p f s", p=DH)
    wq3 = wqT[:].rearrange("(f p) e -> p f e", p=DH)
    wk3 = wkT[:].rearrange("(f p) e -> p f e", p=DH)
    wv3 = wvT[:].rearrange("(f p) e -> p f e", p=DH)

    with TileContext(nc) as tc, nc.allow_low_precision(reason="fp32r ~ fp32"):
        with tc.tile_pool(name="persist", bufs=1) as persist, \
             tc.tile_pool(name="dram", bufs=1, space="DRAM") as dpool:
            tri = persist.tile([DH, DH], ADT, tag="tri")
            kbias = persist.tile([DH, NKT], F32, tag="kbias")
            ones_c = persist.tile([DH, 1], ADT, tag="ones_c")
            ones_r = persist.tile([1, DH], F32R, tag="ones_r")
            ones_ra = persist.tile([1, DH], ADT, tag="ones_ra")
            bq_sb = persist.tile([DH, HPC], F32, tag="bq")
            bk_sb = persist.tile([DH, HPC], F32, tag="bk")
            bv_sb = persist.tile([1, EH], ADT, tag="bv")
            nc.sync.dma_start(tri[:], tri_d[:])
            nc.sync.dma_start(kbias[:], kbias_d[:])
            nc.sync.dma_start(ones_c[:], ones_c_d[:])
            nc.sync.dma_start(ones_r[:], ones_r_d[:])
            nc.sync.dma_start(ones_ra[:], ones_ra_d[:])
            nc.sync.dma_start(bq_sb[:], bq_d[:])
            nc.sync.dma_start(bk_sb[:], bk_d[:])
            nc.sync.dma_start(bv_sb[:], bv_d[:])

            ktS = dpool.tile([EH, S], ADT, tag="ktS")
            vS = dpool.tile([S, EH], ADT, tag="vS")
            ktS3 = ktS[:].rearrange("(et p) s -> p et s", p=DH)

            for _rep in range(reps):
                import contextlib

                repstack = contextlib.ExitStack()
                with repstack:
                    _rep_body(
                        nc, tc, phases, scale, fast, act_dt,
                        kT3, qT3, wk3, wv3, wq3,
                        ktS, ktS3, vS, resid_d, outT_d,
                        tri, kbias, ones_c, ones_r, ones_ra,
                        bq_sb, bk_sb, bv_sb,
                        F32, F32R, ADT, EXP, IDENT,
                    )

    _split_excess_waits(nc, max_waits=1, compute_max_waits=COMPUTE_MAX_WAITS)
    _CACHE[key] = nc
    return nc


def _rep_body(
    nc, tc, phases, scale, fast, act_dt,
    kT3, qT3, wk3, wv3, wq3,
    ktS, ktS3, vS, resid_d, outT_d,
    tri, kbias, ones_c, ones_r, ones_ra,
    bq_sb, bk_sb, bv_sb,
    F32, F32R, ADT, EXP, IDENT,
):
    if True:
                # ---------------- Phase KV ----------------
                # KT[e, s] = (keys @ Wk.T).T and V[s, e] = keys @ Wv.T + bv
                # -> DRAM scratch. bf16 fits both weights resident -> one
                # fused pass over kT; f32r needs a separate V pass.
                fused_v = act_dt == "bf16"
                if "k" in phases:
                    import contextlib

                    kvstack = contextlib.ExitStack()
                    with kvstack:
                        wkp = kvstack.enter_context(tc.tile_pool(name="wk", bufs=1))
                        kcp = kvstack.enter_context(tc.tile_pool(name="kc", bufs=2))
                        stp = kvstack.enter_context(tc.tile_pool(name="kst", bufs=2))
                        pkp = kvstack.enter_context(
                            tc.tile_pool(name="pk", bufs=3, space="PSUM")
                        )
                        if fused_v:
                            wvp = kvstack.enter_context(
                                tc.tile_pool(name="wv", bufs=1)
                            )
                            vstp = kvstack.enter_context(
                                tc.tile_pool(name="vst", bufs=3)
                            )
                            pvp = kvstack.enter_context(
                                tc.tile_pool(name="pv", bufs=3, space="PSUM")
                            )
                        wk_t = wkp.tile([DH, NF * EH], ADT, tag="wk", name="wk")
                        nc.sync.dma_start(
                            wk_t[:].rearrange("p (f e) -> p f e", f=NF), wk3
                        )
                        if fused_v:
                            wv_t = wvp.tile([DH, NF * EH], ADT, tag="wv", name="wv")
                            nc.sync.dma_start(
                                wv_t[:].rearrange("p (f e) -> p f e", f=NF), wv3
                            )
                        for sc in range(NPC * scale.get("k", 1)):
                            s0 = (sc % NPC) * PC
                            kc = kcp.tile([DH, NF * PC], ADT, tag="kc", name="kc")
                            nc.sync.dma_start(
                                kc[:].rearrange("p (f s) -> p f s", f=NF),
                                kT3[:, :, s0 : s0 + PC],
                            )
                            kst = stp.tile(
                                [DH, HPC * PC], ADT, tag="kst", name="kst"
                            )
                            for et in range(HPC):
                                pk = pkp.tile([DH, PC], F32, name="pk")
                                for f in range(NF):
                                    nc.tensor.matmul(
                                        pk[:],
                                        wk_t[
                                            :,
                                            f * EH + et * DH : f * EH + (et + 1) * DH,
                                        ],
                                        kc[:, f * PC : (f + 1) * PC],
                                        start=(f == 0),
                                        stop=(f == NF - 1),
                                    )
                                nc.scalar.activation(
                                    kst[:, et * PC : (et + 1) * PC],
                                    pk[:],
                                    IDENT,
                                    bias=bk_sb[:, et : et + 1],
                                )
                            nc.sync.dma_start(
                                ktS3[:, :, s0 : s0 + PC],
                                kst[:].rearrange("p (et s) -> p et s", et=HPC),
                            )
                            if fused_v:
                                _v_proj_chunk(
                                    nc, s0, kc, wv_t, vstp, pvp, vS,
                                    ones_ra, bv_sb, F32, ADT,
                                )

                # standalone V pass (f32r: weights don't fit together)
                if "v" in phases and not fused_v:
                    with tc.tile_pool(name="wv", bufs=1) as wvp, \
                         tc.tile_pool(name="kc2", bufs=2) as kcp2, \
                         tc.tile_pool(name="vst", bufs=3) as vstp, \
                         tc.tile_pool(name="pv", bufs=3, space="PSUM") as pvp:
                        wv_t = wvp.tile([DH, NF * EH], ADT, tag="wv", name="wv")
                        nc.sync.dma_start(
                            wv_t[:].rearrange("p (f e) -> p f e", f=NF), wv3
                        )
                        for sc in range(NPC * scale.get("v", 1)):
                            s0 = (sc % NPC) * PC
                            kc = kcp2.tile(
                                [DH, NF * PC], ADT, tag="kc2", name="kc2"
                            )
                            nc.sync.dma_start(
                                kc[:].rearrange("p (f s) -> p f s", f=NF),
                                kT3[:, :, s0 : s0 + PC],
                            )
                            _v_proj_chunk(
                                nc, s0, kc, wv_t, vstp, pvp, vS,
                                ones_ra, bv_sb, F32, ADT,
                            )

                # ---------------- Phase Q (QT stays resident) ----------------
                with tc.tile_pool(name="qt", bufs=1) as qtp:
                    qt_t = [
                        qtp.tile([DH, S], ADT, tag=f"qt{et}", name=f"qt{et}")
                        for et in range(HPC)
                    ]
                    if "q" in phases:
                        with tc.tile_pool(name="wq", bufs=1) as wqp, \
                             tc.tile_pool(name="qc", bufs=2) as qcp, \
                             tc.tile_pool(name="pq", bufs=4, space="PSUM") as pqp:
                            wq_t = wqp.tile(
                                [DH, NF * EH], ADT, tag="wq", name="wq"
                            )
                            nc.sync.dma_start(
                                wq_t[:].rearrange("p (f e) -> p f e", f=NF), wq3
                            )
                            for sc in range(NPC * scale.get("q", 1)):
                                s0 = (sc % NPC) * PC
                                qch = qcp.tile(
                                    [DH, NF * PC], ADT, tag="qch", name="qch"
                                )
                                nc.sync.dma_start(
                                    qch[:].rearrange("p (f s) -> p f s", f=NF),
                                    qT3[:, :, s0 : s0 + PC],
                                )
                                for et in range(HPC):
                                    pq = pqp.tile([DH, PC], F32, name="pq")
                                    for f in range(NF):
                                        nc.tensor.matmul(
                                            pq[:],
                                            wq_t[
                                                :,
                                                f * EH
                                                + et * DH : f * EH
                                                + (et + 1) * DH,
                                            ],
                                            qch[:, f * PC : (f + 1) * PC],
                                            start=(f == 0),
                                            stop=(f == NF - 1),
                                        )
                                    nc.scalar.activation(
                                        qt_t[et][:, s0 : s0 + PC],
                                        pq[:],
                                        IDENT,
                                        bias=bq_sb[:, et : et + 1],
                                    )

                    # ---------------- Phase attention ----------------
                    if "attn" in phases:
                        _attention(
                            nc, tc, fast, qt_t, ktS, vS, resid_d, outT_d,
                            tri, kbias, ones_c, ones_r, F32, F32R, ADT, EXP,
                            scale.get("attn", 1),
                        )


def _v_proj_chunk(nc, s0, kc, wv_t, vstp, pvp, vS, ones_ra, bv_sb, F32, ADT):
    for sti in range(PC // DH):
        vst = vstp.tile([DH, EH], ADT, tag="vst", name="vst")
        for ec in range(EH // QC):
            pv = pvp.tile([DH, QC], F32, name="pv")
            for f in range(NF):
                nc.tensor.matmul(
                    pv[:],
                    kc[:, f * PC + sti * DH : f * PC + (sti + 1) * DH],
                    wv_t[:, f * EH + ec * QC : f * EH + (ec + 1) * QC],
                    start=(f == 0),
                    stop=False,
                )
            nc.tensor.matmul(
                pv[:],
                ones_ra[:],
                bv_sb[:, ec * QC : (ec + 1) * QC],
                start=False,
                stop=True,
            )
            nc.scalar.copy(vst[:, ec * QC : (ec + 1) * QC], pv[:])
        nc.sync.dma_start(vS[s0 + sti * DH : s0 + (sti + 1) * DH, :], vst[:])


def _attention(
    nc, tc, fast, qt_t, ktS, vS, resid_d, outT_d,
    tri, kbias, ones_c, ones_r, F32, F32R, ADT, EXP, attn_scale=1,
):
    with tc.tile_pool(name="kvh", bufs=2) as kvhp, \
         tc.tile_pool(name="ex", bufs=4) as exp_p, \
         tc.tile_pool(name="tail", bufs=2) as tailp, \
         tc.tile_pool(name="outs", bufs=2) as outp_sb, \
         tc.tile_pool(name="ps_s", bufs=3, space="PSUM") as pss, \
         tc.tile_pool(name="ps_o", bufs=2, space="PSUM") as pso, \
         tc.tile_pool(name="ps_t", bufs=2, space="PSUM") as pst, \
         tc.tile_pool(name="ps_b", bufs=1, space="PSUM") as psb:
        for hh in range(HPC * attn_scale):
            h = hh % HPC
            ktH = kvhp.tile([DH, S], ADT, tag="ktH", name="ktH")
            nc.sync.dma_start(ktH[:], ktS[h * DH : (h + 1) * DH, :])
            vH = kvhp.tile([DH, S], ADT, tag="vH", name="vH")
            nc.sync.dma_start(
                vH[:].rearrange("p (kt d) -> p kt d", kt=NKT),
                vS[:, h * DH : (h + 1) * DH].rearrange("(kt p) d -> p kt d", p=DH),
            )
            rsd = outp_sb.tile([DH, S], F32, tag="rsd", name="rsd")
            nc.sync.dma_start(rsd[:], resid_d[h * DH : (h + 1) * DH, :])
            oth = outp_sb.tile([DH, S], F32, tag="oth", name="oth")
            for qc in range(NQC):
                q0 = qc * QC
                nkt = 4 * qc + 4
                nfull = 4 * qc  # full (non-band) k-tiles
                po = pso.tile([DH, QC], F32, name="po")
                psum = pst.tile([1, QC], F32, name="psum")

                def pv_sum(kt, exs, off):
                    nc.tensor.matmul(
                        po[:, off:QC],
                        vH[:, kt * DH : (kt + 1) * DH],
                        exs,
                        start=(kt == 0),
                        stop=(kt == nkt - 1),
                    )
                    nc.tensor.matmul(
                        psum[:, off:QC],
                        ones_c[:],
                        exs,
                        start=(kt == 0),
                        stop=(kt == nkt - 1),
                    )

                for kt in range(nfull):
                    ps = pss.tile([DH, QC], F32, name="ps")
                    nc.tensor.matmul(
                        ps[:],
                        ktH[:, kt * DH : (kt + 1) * DH],
                        qt_t[h][:, q0 : q0 + QC],
                        start=True,
                        stop=True,
                    )
                    ex = exp_p.tile([DH, QC], ADT, tag="ex", name="ex")
                    if fast:
                        nc.scalar.activation(ex[:], ps[:], EXP, scale=float(SCALE))
                    else:
                        nc.scalar.activation(
                            ex[:], ps[:], EXP,
                            bias=kbias[:, kt : kt + 1], scale=float(SCALE),
                        )
                    pv_sum(kt, ex[:], 0)
                # diagonal band: tile j's valid q-cols start at j*128
                for j in range(4):
                    kt = nfull + j
                    off = j * DH
                    ps = pss.tile([DH, QC], F32, name="ps")
                    nc.tensor.matmul(
                        ps[:, off:QC],
                        ktH[:, kt * DH : (kt + 1) * DH],
                        qt_t[h][:, q0 + off : q0 + QC],
                        start=True,
                        stop=True,
                    )
                    ex = exp_p.tile([DH, QC], ADT, tag="ex", name="ex")
                    if fast:
                        nc.scalar.activation(
                            ex[:, off:QC], ps[:, off:QC], EXP, scale=float(SCALE)
                        )
                    else:
                        nc.scalar.activation(
                            ex[:, off:QC], ps[:, off:QC], EXP,
                            bias=kbias[:, kt : kt + 1], scale=float(SCALE),
                        )
                    # causal triangle on the diagonal 128x128 block
                    nc.vector.tensor_mul(
                        ex[:, off : off + DH], ex[:, off : off + DH], tri[:]
                    )
                    pv_sum(kt, ex[:, off:QC], off)
                rec = tailp.tile([1, QC], F32R, tag="rec", name="rec")
                nc.vector.reciprocal(rec[:], psum[:])
                pbc = psb.tile([DH, QC], F32, name="pbc")
                nc.tensor.matmul(pbc[:], ones_r[:], rec[:], start=True, stop=True)
                bcs = tailp.tile([DH, QC], F32, tag="bcs", name="bcs")
                nc.vector.tensor_copy(bcs[:], pbc[:])
                nc.vector.tensor_mul(oth[:, q0 : q0 + QC], po[:], bcs[:])
                nc.vector.tensor_add(
                    oth[:, q0 : q0 + QC],
                    oth[:, q0 : q0 + QC],
                    rsd[:, q0 : q0 + QC],
                )
            nc.sync.dma_start(outT_d[h * DH : (h + 1) * DH, :], oth[:])


def _host_prep(queries, keys, Wq, bq, Wk, bk, Wv, bv, act_dt=None):
    """Build the 8 per-core input maps (host-side shard + layout prep)."""
    if act_dt is None:
        act_dt = ACT_DT
    if act_dt == "bf16":
        import ml_dtypes

        adt = ml_dtypes.bfloat16
    else:
        adt = np.float32
    queries = np.ascontiguousarray(queries, dtype=np.float32)
    keys = np.ascontiguousarray(keys, dtype=np.float32)

    qT = np.ascontiguousarray(queries.transpose(0, 2, 1))  # [B, HID, S]
    kT = np.ascontiguousarray(keys.transpose(0, 2, 1))
    qTa = qT.astype(adt)
    kTa = kT.astype(adt)
    WqT = np.ascontiguousarray(np.asarray(Wq, np.float32).T).astype(adt)  # [f, e]
    WkT = np.ascontiguousarray(np.asarray(Wk, np.float32).T).astype(adt)
    WvT = np.ascontiguousarray(np.asarray(Wv, np.float32).T).astype(adt)
    bq = np.asarray(bq, np.float32)
    bk = np.asarray(bk, np.float32)
    bv = np.asarray(bv, np.float32)

    # key padding mask -> additive bias per (b, k): 0 keep, -1e30 mask
    ksum = keys.sum(axis=-1)  # [B, S]
    kbias_all = np.where(ksum != 0.0, np.float32(0), NEG_BIAS).astype(np.float32)

    # causal triangle for the diagonal 128x128 blocks: keep iff q_local >= k_local
    tri = (np.arange(DH)[None, :] >= np.arange(DH)[:, None]).astype(adt)

    ones_c = np.ones((DH, 1), adt)
    ones_r = np.ones((1, DH), np.float32)

    in_maps = []
    for c in range(NCORES):
        b, hg = divmod(c, 2)
        e0 = hg * EH
        in_maps.append(
            {
                "qT": qTa[b],
                "kT": kTa[b],
                "wqT": np.ascontiguousarray(WqT[:, e0 : e0 + EH]),
                "wkT": np.ascontiguousarray(WkT[:, e0 : e0 + EH]),
                "wvT": np.ascontiguousarray(WvT[:, e0 : e0 + EH]),
                "bq_d": np.ascontiguousarray(bq[e0 : e0 + EH].reshape(HPC, DH).T),
                "bk_d": np.ascontiguousarray(bk[e0 : e0 + EH].reshape(HPC, DH).T),
                "bv_d": np.ascontiguousarray(bv[e0 : e0 + EH].reshape(1, EH)).astype(adt),
                "kbias_d": np.ascontiguousarray(kbias_all[b].reshape(NKT, DH).T),
                "tri_d": tri,
                "ones_c_d": ones_c,
                "ones_r_d": ones_r,
                "ones_ra_d": ones_r.astype(adt),
                "resid_d": np.ascontiguousarray(qT[b][e0 : e0 + EH, :]),
            }
        )
    return in_maps


def _assemble(results):
    """results: list of 8 dicts with outT_d [EH, S] -> full [B, S, HID]."""
    out = np.empty((B, S, HID), np.float32)
    for c in range(NCORES):
        b, hg = divmod(c, 2)
        out[b, :, hg * EH : (hg + 1) * EH] = results[c]["outT_d"].T
    return out


def kernel(**inputs):
    from concourse.bass_utils import run_bass_kernel_spmd

    # fast path is valid unless some key row is exactly zero-sum (padding)
    keys = np.asarray(inputs["keys"], np.float32)
    fast = not bool(np.any(keys.sum(axis=-1) == 0.0))
    nc = _build(fast=fast, act_dt=ACT_DT)
    in_maps = _host_prep(**inputs, act_dt=ACT_DT)
    res = run_bass_kernel_spmd(nc, in_maps, core_ids=list(range(NCORES)))
    kernel.last_results = res
    return _assemble(res.results)



# revision 6
# speedup vs baseline: 1.3649x; 1.3649x over previous
"""Trainium2 Bass kernel for nn_MultiHeadAttention_88210038326473.

Reference computation (B=4, S=2048, HID=2048, H=16, DH=128):
    Q = queries @ Wq.T + bq ; K = keys @ Wk.T + bk ; V = keys @ Wv.T + bv
    per-head scores = Qh Kh^T / sqrt(HID), key-padding + causal mask,
    softmax, out = attn @ Vh, concat heads, + queries residual.

Sharding: 8 cores = 4 batches x 2 head-groups (8 heads each). Each core
computes out[b, :, hg*1024:(hg+1)*1024] (stored transposed [1024, 2048];
host transposes back and assembles).

Implementation: fp8(e4m3) operands with DoubleRow matmuls (2 contraction
subtiles of 128 per PE stream) for the three projections and the
attention AV / row-sum matmuls. Host prescales weights by 32 so fp8
weight values avoid the subnormal range; the 1/32 factors are folded
into the exp scale and the row-sum ones value. All of KT/VT/QT stay
SBUF-resident between projection and attention (no DRAM scratch).
Scores are computed transposed (sT[k,q]) in 2-bank PSUM pair tiles,
exp'd in one ScalarE call per pair (fp8 out), causal-masked with
precomputed sliding-window 0/1 masks on DVE, then consumed by
DoubleRow AV and row-sum matmuls. Normalization: DR ones-matmul row
sums -> DVE reciprocal -> PE rank-1 broadcast matmul (f32r) -> DVE
normalize + residual(bf16) add, deferred one q-chunk to keep PE fed.
Q-projection of head h+1 is woven between attention pairs of head h so
ScalarE exp time hides under PE matmuls.
"""

import math

import numpy as np

B, S, HID, H, DH = 4, 2048, 2048, 16, 128
NCORES = 8
HPC = 8          # heads per core
EH = HPC * DH    # 1024 e-dims per core
SCALE = 1.0 / math.sqrt(HID)
WS = 32.0        # host-side weight scale (fp8 subnormal avoidance)
EFF_SCALE = float(SCALE / (WS * WS))  # exp scale: undo Q,K weight scaling
PC = 512         # projection s-chunk (matmul moving N)
NPC = S // PC    # 4
QC = 512         # attention q-chunk
NQC = S // QC    # 4
NKT = S // DH    # 16 k-tiles
NF = HID // DH   # 16 f-tiles (contraction)
NFP = NF // 2    # 8 f-pairs (DoubleRow)
NEG_BIAS = np.float32(-1.0e30)
COMPUTE_MAX_WAITS = 1  # waits allowed on non-CTRL instructions before splitting


CTRL_OPS = ("InstDrain", "InstNoOp", "InstEventSemaphore", "InstISA")


def _split_excess_waits(nc, max_waits=1, compute_max_waits=None):
    """walrus in this container rejects >1 sem-wait per CTRL-class instruction.
    Move excess waits onto preceding NoOps on the same engine. Compute-class
    instructions may support more waits (compute_max_waits)."""
    import concourse.mybir as mybir

    if compute_max_waits is None:
        compute_max_waits = max_waits
    n_split = 0
    for fn in nc.m.functions:
        for blk in fn.blocks:
            insts = list(blk.instructions)
            out = []
            changed = False
            for ins in insts:
                lim = (
                    max_waits
                    if type(ins).__name__ in CTRL_OPS
                    else compute_max_waits
                )
                si = ins.sync_info
                if si is not None and si.on_wait and len(si.on_wait) > lim:
                    waits = list(si.on_wait)
                    carriers, rest = waits[:-lim], waits[-lim:]
                    for i in range(0, len(carriers), max_waits):
                        chunk = carriers[i : i + max_waits]
                        out.append(
                            mybir.InstNoOp(
                                name=f"{ins.name}-ws{i}",
                                engine=ins.engine,
                                bass_nofuse=True,
                                sync_info=mybir.SyncInfo(on_wait=chunk, on_update=[]),
                            )
                        )
                        n_split += 1
                    ins.sync_info = mybir.SyncInfo(
                        on_wait=rest, on_update=list(si.on_update)
                    )
                    changed = True
                out.append(ins)
            if changed:
                blk.instructions = out
    return n_split


_CACHE = {}


def _build(fast=True, zero_bias=True, phases=("kv", "qattn"), reps=1,
           act_dt=None, scale=None):
    """Build the (core-uniform) Bass program. Returns nc.

    fast=True drops the key-padding bias from the exp (valid when no key is
    padding -- checked on host). zero_bias=True skips bias adds (all-zero
    biases, checked on host). reps/scale repeat phases for timing
    instrumentation. act_dt accepted for interface compat (ignored; fp8)."""
    scale = scale or {}
    key = ("nc", fast, zero_bias, tuple(phases), reps,
           tuple(sorted(scale.items())))
    if key in _CACHE:
        return _CACHE[key]

    import concourse.bass as bass
    import concourse.mybir as mybir
    from concourse.tile import TileContext

    F32 = mybir.dt.float32
    F32R = mybir.dt.float32r
    BF16 = mybir.dt.bfloat16
    FP8 = mybir.dt.float8e4
    EXP = mybir.ActivationFunctionType.Exp
    IDENT = mybir.ActivationFunctionType.Identity
    DR = mybir.MatmulPerfMode.DoubleRow

    nc = bass.Bass("TRN2", target_bir_lowering=False, debug=False)

    qT = nc.dram_tensor("qT", [HID, S], FP8, kind="ExternalInput")
    kT = nc.dram_tensor("kT", [HID, S], FP8, kind="ExternalInput")
    wqT = nc.dram_tensor("wqT", [HID, EH], FP8, kind="ExternalInput")
    wkT = nc.dram_tensor("wkT", [HID, EH], FP8, kind="ExternalInput")
    wvT = nc.dram_tensor("wvT", [HID, EH], FP8, kind="ExternalInput")
    bq_d = nc.dram_tensor("bq_d", [DH, HPC], F32, kind="ExternalInput")
    bk_d = nc.dram_tensor("bk_d", [DH, HPC], F32, kind="ExternalInput")
    bv_d = nc.dram_tensor("bv_d", [1, EH], FP8, kind="ExternalInput")
    kbias_d = nc.dram_tensor("kbias_d", [DH, NKT], F32, kind="ExternalInput")
    wins_d = nc.dram_tensor("wins_d", [DH, 4 * QC], FP8, kind="ExternalInput")
    ones16_d = nc.dram_tensor("ones16_d", [DH, 32], FP8, kind="ExternalInput")
    onesr32_d = nc.dram_tensor("onesr32_d", [1, DH], F32R, kind="ExternalInput")
    onesr8_d = nc.dram_tensor("onesr8_d", [1, DH], FP8, kind="ExternalInput")
    resid_d = nc.dram_tensor("resid_d", [EH, S], BF16, kind="ExternalInput")
    outT_d = nc.dram_tensor("outT_d", [EH, S], F32, kind="ExternalOutput")

    # 3D views with the 128-partition dim innermost on rows
    qT3 = qT[:].rearrange("(f p) s -> p f s", p=DH)
    kT3 = kT[:].rearrange("(f p) s -> p f s", p=DH)
    wq3 = wqT[:].rearrange("(f p) e -> p f e", p=DH)
    wk3 = wkT[:].rearrange("(f p) e -> p f e", p=DH)
    wv3 = wvT[:].rearrange("(f p) e -> p f e", p=DH)

    ctx = dict(
        F32=F32, F32R=F32R, BF16=BF16, FP8=FP8, EXP=EXP, IDENT=IDENT, DR=DR,
        fast=fast, zero_bias=zero_bias, scale=scale,
        qT3=qT3, kT3=kT3, wq3=wq3, wk3=wk3, wv3=wv3,
        resid_d=resid_d, outT_d=outT_d,
    )

    with TileContext(nc) as tc, nc.allow_low_precision(reason="fp8 attn"):
        with tc.tile_pool(name="persist", bufs=1) as persist:
            kres = persist.tile([DH, HPC * S], FP8, tag="kres")
            qres = persist.tile([DH, HPC * S], FP8, tag="qres")
            vres = persist.tile([DH, NKT * EH], FP8, tag="vres")
            wins = persist.tile([DH, 4 * QC], FP8, tag="wins")
            ones16 = persist.tile([DH, 32], FP8, tag="ones16")
            onesr32 = persist.tile([1, DH], F32R, tag="onesr32")
            onesr8 = persist.tile([1, DH], FP8, tag="onesr8")
            bq_sb = persist.tile([DH, HPC], F32, tag="bq")
            bk_sb = persist.tile([DH, HPC], F32, tag="bk")
            bv_sb = persist.tile([1, EH], FP8, tag="bv")
            kbias = persist.tile([DH, NKT], F32, tag="kbias")
            nc.sync.dma_start(wins[:], wins_d[:])
            nc.sync.dma_start(ones16[:], ones16_d[:])
            nc.sync.dma_start(onesr32[:], onesr32_d[:])
            nc.sync.dma_start(onesr8[:], onesr8_d[:])
            nc.sync.dma_start(bq_sb[:], bq_d[:])
            nc.sync.dma_start(bk_sb[:], bk_d[:])
            nc.sync.dma_start(bv_sb[:], bv_d[:])
            nc.sync.dma_start(kbias[:], kbias_d[:])

            ctx.update(
                kres3=kres[:].rearrange("p (h s) -> p h s", h=HPC),
                qres3=qres[:].rearrange("p (h s) -> p h s", h=HPC),
                vres3=vres[:].rearrange("p (kt e) -> p kt e", kt=NKT),
                wins=wins, ones16=ones16, onesr32=onesr32, onesr8=onesr8,
                bq_sb=bq_sb, bk_sb=bk_sb, bv_sb=bv_sb, kbias=kbias,
            )

            for _rep in range(reps):
                _rep_body(nc, tc, phases, ctx)

    _split_excess_waits(nc, max_waits=1, compute_max_waits=COMPUTE_MAX_WAITS)
    _CACHE[key] = nc
    return nc


def _proj_copy(nc, ctx, dst, psrc, bias_col):
    """PSUM [DH, 512] f32 -> SBUF fp8, optional per-partition bias."""
    if ctx["zero_bias"]:
        nc.vector.tensor_copy(dst, psrc)
    else:
        nc.scalar.activation(dst, psrc, ctx["IDENT"], bias=bias_col)


def _rep_body(nc, tc, phases, ctx):
    import contextlib

    DR = ctx["DR"]
    F32 = ctx["F32"]
    FP8 = ctx["FP8"]
    scale = ctx["scale"]

    if "kv" in phases:
        with tc.tile_pool(name="wk", bufs=1) as wkp, \
             tc.tile_pool(name="wv", bufs=1) as wvp, \
             tc.tile_pool(name="kc", bufs=2) as kcp, \
             tc.tile_pool(name="pk", bufs=3, space="PSUM") as pkp, \
             tc.tile_pool(name="pv", bufs=3, space="PSUM") as pvp:
            wk_t = wkp.tile([DH, NF * EH], FP8, tag="wk", name="wk")
            nc.sync.dma_start(
                wk_t[:].rearrange("p (f e) -> p f e", f=NF), ctx["wk3"]
            )
            wv_t = wvp.tile([DH, NF * EH], FP8, tag="wv", name="wv")
            nc.sync.dma_start(
                wv_t[:].rearrange("p (f e) -> p f e", f=NF), ctx["wv3"]
            )
            wk3t = wk_t[:].rearrange("p (f e) -> p f e", f=NF)
            wv3t = wv_t[:].rearrange("p (f e) -> p f e", f=NF)
            for sc in range(NPC * scale.get("kv", 1)):
                s0 = (sc % NPC) * PC
                kc = kcp.tile([DH, NF * PC], FP8, tag="kc", name="kc")
                kc3 = kc[:].rearrange("p (f s) -> p f s", f=NF)
                nc.sync.dma_start(kc3, ctx["kT3"][:, :, s0 : s0 + PC])
                for et in range(HPC):
                    pk = pkp.tile([DH, PC], F32, name="pk")
                    for i in range(NFP):
                        nc.tensor.matmul(
                            pk[:],
                            wk3t[:, 2 * i : 2 * i + 2, et * DH : (et + 1) * DH],
                            kc3[:, 2 * i : 2 * i + 2, :],
                            start=(i == 0), stop=(i == NFP - 1), perf_mode=DR,
                        )
                    _proj_copy(
                        nc, ctx, ctx["kres3"][:, et, s0 : s0 + PC], pk[:],
                        ctx["bk_sb"][:, et : et + 1],
                    )
                for sti in range(PC // DH):
                    kt = s0 // DH + sti
                    for ec in range(2):
                        pv = pvp.tile([DH, 512], F32, name="pv")
                        for i in range(NFP):
                            nc.tensor.matmul(
                                pv[:],
                                kc3[:, 2 * i : 2 * i + 2,
                                    sti * DH : (sti + 1) * DH],
                                wv3t[:, 2 * i : 2 * i + 2,
                                     ec * 512 : (ec + 1) * 512],
                                start=(i == 0),
                                stop=(ctx["zero_bias"] and i == NFP - 1),
                                perf_mode=DR,
                            )
                        if not ctx["zero_bias"]:
                            nc.tensor.matmul(
                                pv[:],
                                ctx["onesr8"][:],
                                ctx["bv_sb"][:, ec * 512 : (ec + 1) * 512],
                                start=False, stop=True,
                            )
                        nc.vector.tensor_copy(
                            ctx["vres3"][:, kt, ec * 512 : (ec + 1) * 512], pv[:]
                        )

    if "qattn" in phases:
        with tc.tile_pool(name="wq", bufs=1) as wqp, \
             tc.tile_pool(name="qall", bufs=1) as qallp, \
             tc.tile_pool(name="ex", bufs=3) as expp, \
             tc.tile_pool(name="tailsb", bufs=2) as tailsb, \
             tc.tile_pool(name="bcs", bufs=2) as bcsp, \
             tc.tile_pool(name="rsd", bufs=2) as rsdp, \
             tc.tile_pool(name="oth", bufs=3) as othp, \
             tc.tile_pool(name="psm", bufs=2, space="PSUM") as psm, \
             tc.tile_pool(name="po", bufs=2, space="PSUM") as pop, \
             tc.tile_pool(name="psums", bufs=1, space="PSUM") as psumsp, \
             tc.tile_pool(name="pbc", bufs=1, space="PSUM") as pbcp:
            wq_t = wqp.tile([DH, NF * EH], FP8, tag="wq", name="wq")
            nc.sync.dma_start(
                wq_t[:].rearrange("p (f e) -> p f e", f=NF), ctx["wq3"]
            )
            wq3t = wq_t[:].rearrange("p (f e) -> p f e", f=NF)
            qall = qallp.tile([DH, NF * S], FP8, tag="qall", name="qall")
            qall3 = qall[:].rearrange("p (f s) -> p f s", f=NF)
            nc.sync.dma_start(qall3, ctx["qT3"])

            def emit_qproj(h, sc):
                # same allocated size as the score-pair tiles so the shared
                # untagged slot stays at 2 banks x 2 bufs
                pq_full = psm.tile([DH, 2 * QC], F32, tag="pp", name="pq")
                pq = pq_full[:, 0:PC]
                for i in range(NFP):
                    nc.tensor.matmul(
                        pq,
                        wq3t[:, 2 * i : 2 * i + 2, h * DH : (h + 1) * DH],
                        qall3[:, 2 * i : 2 * i + 2, sc * PC : (sc + 1) * PC],
                        start=(i == 0), stop=(i == NFP - 1), perf_mode=DR,
                    )
                _proj_copy(
                    nc, ctx, ctx["qres3"][:, h, sc * PC : (sc + 1) * PC], pq,
                    ctx["bq_sb"][:, h : h + 1],
                )

            nheads = HPC * scale.get("attn", 1)
            # head 0's Q projection up front
            for sc in range(NPC):
                emit_qproj(0, sc)

            pending_tail = [None]

            def flush_tail():
                if pending_tail[0] is not None:
                    pending_tail[0]()
                    pending_tail[0] = None

            ones16v = ctx["ones16"][:].rearrange("p (k m) -> p k m", k=2)

            for hh in range(nheads):
                h = hh % HPC
                hn = (hh + 1) % HPC if hh + 1 < nheads else None
                rsd = rsdp.tile([DH, S], ctx["BF16"], tag="rsd", name="rsd")
                nc.sync.dma_start(
                    rsd[:], ctx["resid_d"][h * DH : (h + 1) * DH, :]
                )
                # weave points: next head's 4 q-proj chunks spread over this
                # head's attention pairs (20 pairs -> after pairs 2,7,12,17)
                weave = {2: 0, 7: 1, 12: 2, 17: 3} if hn is not None else {}
                pair_no = 0
                for qcI in range(NQC):
                    q0 = qcI * QC
                    nkt = 4 * (qcI + 1)
                    npair = nkt // 2
                    po = pop.tile([DH, QC], F32, name="po")
                    psums = psumsp.tile([1, QC], F32, name="psums")
                    oth = othp.tile([DH, QC], F32, tag="oth", name="oth")
                    for pi in range(npair):
                        kt0 = 2 * pi
                        pp = psm.tile([DH, 2 * QC], F32, tag="pp", name="pp")
                        ppv = pp[:].rearrange("p (j q) -> p j q", j=2)
                        for j in range(2):
                            kt = kt0 + j
                            nc.tensor.matmul(
                                ppv[:, j, :],
                                ctx["kres3"][:, h, kt * DH : (kt + 1) * DH],
                                ctx["qres3"][:, h, q0 : q0 + QC],
                                start=True, stop=True,
                            )
                        ex = expp.tile([DH, 2 * QC], FP8, tag="ex", name="ex")
                        if ctx["fast"]:
                            nc.scalar.activation(
                                ex[:], pp[:], ctx["EXP"], scale=EFF_SCALE
                            )
                        else:
                            ex3b = ex[:].rearrange("p (j q) -> p j q", j=2)
                            for j in range(2):
                                kt = kt0 + j
                                nc.scalar.activation(
                                    ex3b[:, j, :], ppv[:, j, :], ctx["EXP"],
                                    bias=ctx["kbias"][:, kt : kt + 1],
                                    scale=EFF_SCALE,
                                )
                        if kt0 >= nkt - 4:
                            jb0 = kt0 - (nkt - 4)
                            nc.vector.tensor_mul(
                                ex[:], ex[:],
                                ctx["wins"][:, jb0 * QC : (jb0 + 2) * QC],
                            )
                        ex3 = ex[:].rearrange("p (k q) -> p k q", k=2)
                        nc.tensor.matmul(
                            po[:],
                            ctx["vres3"][:, kt0 : kt0 + 2,
                                         h * DH : (h + 1) * DH],
                            ex3,
                            start=(pi == 0), stop=(pi == npair - 1),
                            perf_mode=DR,
                        )
                        nc.tensor.matmul(
                            psums[:],
                            ones16v[:, :, 0:1],
                            ex3,
                            start=(pi == 0), stop=(pi == npair - 1),
                            perf_mode=DR,
                        )
                        if pair_no in weave:
                            emit_qproj(hn, weave[pair_no])
                        pair_no += 1
                    # free psums promptly (copy to SBUF) so next q-chunk's
                    # row-sum matmuls don't wait on the tail
                    sums_sb = tailsb.tile([1, QC], F32, tag="sums", name="sums")
                    nc.vector.tensor_copy(sums_sb[:], psums[:])
                    rec = tailsb.tile([1, QC], ctx["F32R"], tag="rec", name="rec")
                    nc.vector.reciprocal(rec[:], sums_sb[:])
                    flush_tail()

                    def make_tail(po=po, rec=rec, oth=oth, rsd=rsd,
                                  h=h, q0=q0):
                        def tail():
                            pbc = pbcp.tile([DH, QC], F32, name="pbc")
                            nc.tensor.matmul(
                                pbc[:], ctx["onesr32"][:], rec[:],
                                start=True, stop=True,
                            )
                            bcs = bcsp.tile([DH, QC], F32, tag="bcs", name="bcs")
                            nc.vector.tensor_copy(bcs[:], pbc[:])
                            nc.vector.tensor_mul(oth[:], po[:], bcs[:])
                            nc.vector.tensor_add(
                                oth[:], oth[:], rsd[:, q0 : q0 + QC]
                            )
                            nc.sync.dma_start(
                                ctx["outT_d"][h * DH : (h + 1) * DH,
                                              q0 : q0 + QC],
                                oth[:],
                            )
                        return tail

                    pending_tail[0] = make_tail()
            flush_tail()


def _host_prep(queries, keys, Wq, bq, Wk, bk, Wv, bv, act_dt=None):
    """Build the 8 per-core input maps (host-side shard + layout prep)."""
    import ml_dtypes

    fp8 = ml_dtypes.float8_e4m3
    bf16 = ml_dtypes.bfloat16

    def to_fp8(x):
        return np.clip(x, -240.0, 240.0).astype(fp8)

    queries = np.ascontiguousarray(queries, dtype=np.float32)
    keys = np.ascontiguousarray(keys, dtype=np.float32)

    qT = np.ascontiguousarray(queries.transpose(0, 2, 1))  # [B, HID, S]
    kT = np.ascontiguousarray(keys.transpose(0, 2, 1))
    qT8 = to_fp8(qT)
    kT8 = to_fp8(kT)
    WqT = to_fp8(np.asarray(Wq, np.float32).T * WS)  # [f, e]
    WkT = to_fp8(np.asarray(Wk, np.float32).T * WS)
    WvT = to_fp8(np.asarray(Wv, np.float32).T * WS)
    bq = np.asarray(bq, np.float32) * WS
    bk = np.asarray(bk, np.float32) * WS
    bv = np.asarray(bv, np.float32) * WS

    # key padding mask -> additive bias per (b, k): 0 keep, -1e30 mask
    ksum = keys.sum(axis=-1)  # [B, S]
    kbias_all = np.where(ksum != 0.0, np.float32(0), NEG_BIAS).astype(np.float32)

    # sliding-window causal masks for the 4 diagonal-band k-tiles of a
    # q-chunk: wins[p, j*QC + q] = 1 iff q >= p + 128*j
    j_idx = np.arange(4)[None, :, None]
    wins = (
        np.arange(QC)[None, None, :] >= (np.arange(DH)[:, None, None] + DH * j_idx)
    ).astype(fp8).reshape(DH, 4 * QC)

    ones16 = np.full((DH, 32), WS, fp8)       # row-sum DR weights (value 32)
    onesr32 = np.ones((1, DH), np.float32)    # broadcast lhsT (f32r)
    onesr8 = np.ones((1, DH), fp8)            # V-bias lhsT

    in_maps = []
    for c in range(NCORES):
        b, hg = divmod(c, 2)
        e0 = hg * EH
        in_maps.append(
            {
                "qT": qT8[b],
                "kT": kT8[b],
                "wqT": np.ascontiguousarray(WqT[:, e0 : e0 + EH]),
                "wkT": np.ascontiguousarray(WkT[:, e0 : e0 + EH]),
                "wvT": np.ascontiguousarray(WvT[:, e0 : e0 + EH]),
                "bq_d": np.ascontiguousarray(bq[e0 : e0 + EH].reshape(HPC, DH).T),
                "bk_d": np.ascontiguousarray(bk[e0 : e0 + EH].reshape(HPC, DH).T),
                "bv_d": to_fp8(bv[e0 : e0 + EH].reshape(1, EH)),
                "kbias_d": np.ascontiguousarray(kbias_all[b].reshape(NKT, DH).T),
                "wins_d": wins,
                "ones16_d": ones16,
                "onesr32_d": onesr32,
                "onesr8_d": onesr8,
                "resid_d": qT[b][e0 : e0 + EH, :].astype(bf16),
            }
        )
    return in_maps


def _assemble(results):
    """results: list of 8 dicts with outT_d [EH, S] -> full [B, S, HID]."""
    out = np.empty((B, S, HID), np.float32)
    for c in range(NCORES):
        b, hg = divmod(c, 2)
        out[b, :, hg * EH : (hg + 1) * EH] = results[c]["outT_d"].T
    return out


def _flags(inputs):
    keys = np.asarray(inputs["keys"], np.float32)
    fast = not bool(np.any(keys.sum(axis=-1) == 0.0))
    zero_bias = all(
        not np.any(np.asarray(inputs[k], np.float32))
        for k in ("bq", "bk", "bv")
    )
    return fast, zero_bias


def kernel(**inputs):
    from concourse.bass_utils import run_bass_kernel_spmd

    fast, zero_bias = _flags(inputs)
    nc = _build(fast=fast, zero_bias=zero_bias)
    in_maps = _host_prep(**inputs)
    res = run_bass_kernel_spmd(nc, in_maps, core_ids=list(range(NCORES)))
    kernel.last_results = res
    return _assemble(res.results)


# revision 21
# speedup vs baseline: 4.4475x; 3.2585x over previous
"""Trainium2 Bass kernel for nn_MultiHeadAttention_88210038326473.

Reference computation (B=4, S=2048, HID=2048, H=16, DH=128):
    Q = queries @ Wq.T + bq ; K = keys @ Wk.T + bk ; V = keys @ Wv.T + bv
    per-head scores = Qh Kh^T / sqrt(HID), key-padding + causal mask,
    softmax, out = attn @ Vh, concat heads, + queries residual.

Sharding: 8 cores = 4 batches x 2 head-groups (8 heads each). Each core
computes out[b, :, hg*1024:(hg+1)*1024] (stored transposed [1024, 2048];
host transposes back and assembles).

Implementation: fp8(e4m3) operands with DoubleRow matmuls (2 contraction
subtiles of 128 per PE stream) for the three projections and the
attention AV / row-sum matmuls. Host prescales weights by 32 so fp8
weight values avoid the subnormal range; the 1/32 factors are folded
into the exp scale and the row-sum ones value. All of KT/VT/QT stay
SBUF-resident between projection and attention (no DRAM scratch).
Scores are computed transposed (sT[k,q]) in 2-bank PSUM pair tiles,
exp'd in one ScalarE call per pair (fp8 out), causal-masked with
precomputed sliding-window 0/1 masks on DVE, then consumed by
DoubleRow AV and row-sum matmuls. Normalization: DR ones-matmul row
sums -> DVE reciprocal -> PE rank-1 broadcast matmul (f32r) -> DVE
normalize + residual(bf16) add, deferred one q-chunk to keep PE fed.
Q-projection of head h+1 is woven between attention pairs of head h so
ScalarE exp time hides under PE matmuls.
"""

import math
import os as _osmod

import numpy as np

_osenv = _osmod.environ

B, S, HID, H, DH = 4, 2048, 2048, 16, 128
NCORES = 8
HPC = 8          # heads per core
EH = HPC * DH    # 1024 e-dims per core
SCALE = 1.0 / math.sqrt(HID)
WS = 32.0        # host-side weight scale (fp8 subnormal avoidance)
EFF_SCALE = float(SCALE / (WS * WS))  # exp scale: undo Q,K weight scaling
PC = 512         # projection s-chunk (matmul moving N)
NPC = S // PC    # 4
QC = 512         # attention q-chunk
NQC = S // QC    # 4
NKT = S // DH    # 16 k-tiles
NF = HID // DH   # 16 f-tiles (contraction)
NFP = NF // 2    # 8 f-pairs (DoubleRow)
NEG_BIAS = np.float32(-1.0e30)
COMPUTE_MAX_WAITS = int(_osenv.get('K_CMW', '1'))  # waits on non-CTRL instructions


CTRL_OPS = ("InstDrain", "InstNoOp", "InstEventSemaphore", "InstISA")


def _split_excess_waits(nc, max_waits=1, compute_max_waits=None):
    """walrus in this container rejects >1 sem-wait per CTRL-class instruction.
    Move excess waits onto preceding NoOps on the same engine. Compute-class
    instructions may support more waits (compute_max_waits)."""
    import concourse.mybir as mybir

    if compute_max_waits is None:
        compute_max_waits = max_waits
    n_split = 0
    for fn in nc.m.functions:
        for blk in fn.blocks:
            insts = list(blk.instructions)
            out = []
            changed = False
            for ins in insts:
                lim = (
                    max_waits
                    if type(ins).__name__ in CTRL_OPS
                    else compute_max_waits
                )
                si = ins.sync_info
                if si is not None and si.on_wait and len(si.on_wait) > lim:
                    waits = list(si.on_wait)
                    carriers, rest = waits[:-lim], waits[-lim:]
                    for i in range(0, len(carriers), max_waits):
                        chunk = carriers[i : i + max_waits]
                        out.append(
                            mybir.InstNoOp(
                                name=f"{ins.name}-ws{i}",
                                engine=ins.engine,
                                bass_nofuse=True,
                                sync_info=mybir.SyncInfo(on_wait=chunk, on_update=[]),
                            )
                        )
                        n_split += 1
                    ins.sync_info = mybir.SyncInfo(
                        on_wait=rest, on_update=list(si.on_update)
                    )
                    changed = True
                out.append(ins)
            if changed:
                blk.instructions = out
    return n_split


_CACHE = {}


def _build(fast=True, zero_bias=True, phases=("kv", "qattn"), reps=1,
           act_dt=None, scale=None):
    """Build the (core-uniform) Bass program. Returns nc.

    fast=True drops the key-padding bias from the exp (valid when no key is
    padding -- checked on host). zero_bias=True skips bias adds (all-zero
    biases, checked on host). reps/scale repeat phases for timing
    instrumentation. act_dt accepted for interface compat (ignored; fp8)."""
    scale = scale or {}
    key = ("nc", fast, zero_bias, tuple(phases), reps,
           tuple(sorted(scale.items())))
    if key in _CACHE:
        return _CACHE[key]

    import concourse.bass as bass
    import concourse.mybir as mybir
    from concourse.tile import TileContext

    F32 = mybir.dt.float32
    F32R = mybir.dt.float32r
    BF16 = mybir.dt.bfloat16
    FP8 = mybir.dt.float8e4
    EXP = mybir.ActivationFunctionType.Exp
    IDENT = mybir.ActivationFunctionType.Identity
    DR = mybir.MatmulPerfMode.DoubleRow

    nc = bass.Bass("TRN2", target_bir_lowering=False, debug=False)

    qT = nc.dram_tensor("qT", [HID, S], FP8, kind="ExternalInput")
    kT = nc.dram_tensor("kT", [HID, S], FP8, kind="ExternalInput")
    wqT = nc.dram_tensor("wqT", [HID, EH], FP8, kind="ExternalInput")
    wkT = nc.dram_tensor("wkT", [HID, EH], FP8, kind="ExternalInput")
    wvT = nc.dram_tensor("wvT", [HID, EH], FP8, kind="ExternalInput")
    bq_d = nc.dram_tensor("bq_d", [DH, HPC], F32, kind="ExternalInput")
    bk_d = nc.dram_tensor("bk_d", [DH, HPC], F32, kind="ExternalInput")
    bv_d = nc.dram_tensor("bv_d", [1, EH], FP8, kind="ExternalInput")
    kbias_d = nc.dram_tensor("kbias_d", [DH, NKT], F32, kind="ExternalInput")
    wins_d = nc.dram_tensor("wins_d", [DH, 4 * QC], FP8, kind="ExternalInput")
    ones16_d = nc.dram_tensor("ones16_d", [DH, 32], FP8, kind="ExternalInput")
    onesr32_d = nc.dram_tensor("onesr32_d", [1, DH], F32R, kind="ExternalInput")
    onesr8_d = nc.dram_tensor("onesr8_d", [1, DH], FP8, kind="ExternalInput")
    resid_d = nc.dram_tensor("resid_d", [EH, S], BF16, kind="ExternalInput")
    outT_d = nc.dram_tensor("outT_d", [EH, S], F32, kind="ExternalOutput")

    # 3D views with the 128-partition dim innermost on rows
    qT3 = qT[:].rearrange("(f p) s -> p f s", p=DH)
    kT3 = kT[:].rearrange("(f p) s -> p f s", p=DH)
    wq3 = wqT[:].rearrange("(f p) e -> p f e", p=DH)
    wk3 = wkT[:].rearrange("(f p) e -> p f e", p=DH)
    wv3 = wvT[:].rearrange("(f p) e -> p f e", p=DH)

    ctx = dict(
        F32=F32, F32R=F32R, BF16=BF16, FP8=FP8, EXP=EXP, IDENT=IDENT, DR=DR,
        fast=fast, zero_bias=zero_bias, scale=scale,
        qT3=qT3, kT3=kT3, wq3=wq3, wk3=wk3, wv3=wv3,
        resid_d=resid_d, outT_d=outT_d,
    )

    with TileContext(nc) as tc, nc.allow_low_precision(reason="fp8 attn"):
        with tc.tile_pool(name="persist", bufs=1) as persist:
            kres = persist.tile([DH, HPC * S], FP8, tag="kres")
            qres = persist.tile([DH, HPC * S], FP8, tag="qres")
            vres = persist.tile([DH, NKT * EH], FP8, tag="vres")
            wins = persist.tile([DH, 4 * QC], FP8, tag="wins")
            ones16 = persist.tile([DH, 32], FP8, tag="ones16")
            onesr32 = persist.tile([1, DH], F32R, tag="onesr32")
            onesr8 = persist.tile([1, DH], FP8, tag="onesr8")
            bq_sb = persist.tile([DH, HPC], F32, tag="bq")
            bk_sb = persist.tile([DH, HPC], F32, tag="bk")
            bv_sb = persist.tile([1, EH], FP8, tag="bv")
            kbias = persist.tile([DH, NKT], F32, tag="kbias")
            nc.sync.dma_start(wins[:], wins_d[:])
            nc.sync.dma_start(ones16[:], ones16_d[:])
            nc.sync.dma_start(onesr32[:], onesr32_d[:])
            nc.sync.dma_start(onesr8[:], onesr8_d[:])
            nc.sync.dma_start(bq_sb[:], bq_d[:])
            nc.sync.dma_start(bk_sb[:], bk_d[:])
            nc.sync.dma_start(bv_sb[:], bv_d[:])
            nc.sync.dma_start(kbias[:], kbias_d[:])

            ctx.update(
                kres3=kres[:].rearrange("p (h s) -> p h s", h=HPC),
                qres3=qres[:].rearrange("p (h s) -> p h s", h=HPC),
                vres3=vres[:].rearrange("p (kt e) -> p kt e", kt=NKT),
                wins=wins, ones16=ones16, onesr32=onesr32, onesr8=onesr8,
                bq_sb=bq_sb, bk_sb=bk_sb, bv_sb=bv_sb, kbias=kbias,
            )

            for _rep in range(reps):
                _rep_body(nc, tc, phases, ctx)

    _split_excess_waits(nc, max_waits=1, compute_max_waits=COMPUTE_MAX_WAITS)
    _CACHE[key] = nc
    return nc


import os as _os

TRI_ENGINE = _os.environ.get("K_TRI_ENGINE", "vector")
ADD_ENGINE = _os.environ.get("K_ADD_ENGINE", "vector")


def _tri_eng(nc):
    return nc.gpsimd if TRI_ENGINE == "gpsimd" else nc.vector


def _add_eng(nc):
    return nc.gpsimd if ADD_ENGINE == "gpsimd" else nc.vector


def _proj_copy(nc, ctx, dst, psrc, bias_col, engine="vector"):
    """PSUM [DH, 512] f32 -> SBUF fp8, optional per-partition bias."""
    if ctx["zero_bias"]:
        if engine == "scalar":
            nc.scalar.copy(dst, psrc)
        else:
            nc.vector.tensor_copy(dst, psrc)
    else:
        nc.scalar.activation(dst, psrc, ctx["IDENT"], bias=bias_col)


def _rep_body(nc, tc, phases, ctx):
    DR = ctx["DR"]
    F32 = ctx["F32"]
    F32R = ctx["F32R"]
    FP8 = ctx["FP8"]
    scale = ctx["scale"]

    # Wave-structured schedule: for each 512-token chunk c, project K/V for
    # that chunk, then run every head's attention q-chunk c (which only needs
    # k-tiles <= 4c+3, i.e. chunks <= c). This interleaves the ScalarE-heavy
    # exp work with the PE-heavy projections across the whole timeline.
    # PSUM (8 banks): pp-slots 2x2 (scores pairs + K/V/Q proj tiles + the
    # tail broadcast), po 2, psums 1, pbc 1.
    with tc.tile_pool(name="wk", bufs=1) as wkp, \
         tc.tile_pool(name="wv", bufs=1) as wvp, \
         tc.tile_pool(name="wq", bufs=1) as wqp, \
         tc.tile_pool(name="qall", bufs=1) as qallp, \
         tc.tile_pool(name="kc", bufs=2) as kcp, \
         tc.tile_pool(name="ex", bufs=3) as expp, \
         tc.tile_pool(name="tailsb", bufs=2) as tailsb, \
         tc.tile_pool(name="bcs", bufs=2) as bcsp, \
         tc.tile_pool(name="rsd", bufs=3) as rsdp, \
         tc.tile_pool(name="oth", bufs=3) as othp, \
         tc.tile_pool(name="psm", bufs=2, space="PSUM") as psm, \
         tc.tile_pool(name="po", bufs=2, space="PSUM") as pop, \
         tc.tile_pool(name="psums", bufs=1, space="PSUM") as psumsp, \
         tc.tile_pool(name="pbc", bufs=1, space="PSUM") as pbcp:
        wk_t = wkp.tile([DH, NF * EH], FP8, tag="wk", name="wk")
        nc.sync.dma_start(
            wk_t[:].rearrange("p (f e) -> p f e", f=NF), ctx["wk3"]
        )
        wv_t = wvp.tile([DH, NF * EH], FP8, tag="wv", name="wv")
        nc.sync.dma_start(
            wv_t[:].rearrange("p (f e) -> p f e", f=NF), ctx["wv3"]
        )
        wq_t = wqp.tile([DH, NF * EH], FP8, tag="wq", name="wq")
        qall = qallp.tile([DH, NF * S], FP8, tag="qall", name="qall")
        qall3 = qall[:].rearrange("p (f s) -> p f s", f=NF)
        wk3t = wk_t[:].rearrange("p (f e) -> p f e", f=NF)
        wv3t = wv_t[:].rearrange("p (f e) -> p f e", f=NF)
        wq3t = wq_t[:].rearrange("p (f e) -> p f e", f=NF)
        ones16v = ctx["ones16"][:].rearrange("p (k m) -> p k m", k=2)
        tri = ctx["wins"][:, 0:DH]

        # zero both score-PSUM slots once: band pairs exp() regions their
        # matmuls never wrote; stale PSUM must stay finite
        for _z in range(2):
            ppz = psm.tile([DH, 2 * QC], F32, tag="pp", name="ppz")
            nc.vector.memset(ppz[:], 0.0)

        def emit_kv_chunk(s0):
            kc = kcp.tile([DH, NF * PC], FP8, tag="kc", name="kc")
            kc3 = kc[:].rearrange("p (f s) -> p f s", f=NF)
            nc.sync.dma_start(kc3, ctx["kT3"][:, :, s0 : s0 + PC])
            if s0 == 0:
                # q-side loads ride behind the wave-0 K-critical loads
                nc.sync.dma_start(qall3[:, :, 0:PC], ctx["qT3"][:, :, 0:PC])
                nc.sync.dma_start(
                    wq_t[:].rearrange("p (f e) -> p f e", f=NF), ctx["wq3"]
                )
            else:
                nc.sync.dma_start(
                    qall3[:, :, s0 : s0 + PC], ctx["qT3"][:, :, s0 : s0 + PC]
                )
            for et in range(HPC):
                pk_full = psm.tile([DH, 2 * QC], F32, tag="pp", name="pk")
                pk = pk_full[:, 0:PC]
                for i in range(NFP):
                    nc.tensor.matmul(
                        pk,
                        wk3t[:, 2 * i : 2 * i + 2, et * DH : (et + 1) * DH],
                        kc3[:, 2 * i : 2 * i + 2, :],
                        start=(i == 0), stop=(i == NFP - 1), perf_mode=DR,
                    )
                _proj_copy(
                    nc, ctx, ctx["kres3"][:, et, s0 : s0 + PC], pk,
                    ctx["bk_sb"][:, et : et + 1], engine="scalar",
                )
            for sti in range(PC // DH):
                kt = s0 // DH + sti
                for ec in range(2):
                    pv_full = psm.tile([DH, 2 * QC], F32, tag="pp", name="pv")
                    pv = pv_full[:, 0:512]
                    for i in range(NFP):
                        nc.tensor.matmul(
                            pv,
                            kc3[:, 2 * i : 2 * i + 2,
                                sti * DH : (sti + 1) * DH],
                            wv3t[:, 2 * i : 2 * i + 2,
                                 ec * 512 : (ec + 1) * 512],
                            start=(i == 0),
                            stop=(ctx["zero_bias"] and i == NFP - 1),
                            perf_mode=DR,
                        )
                    if not ctx["zero_bias"]:
                        nc.tensor.matmul(
                            pv,
                            ctx["onesr8"][:],
                            ctx["bv_sb"][:, ec * 512 : (ec + 1) * 512],
                            start=False, stop=True,
                        )
                    nc.scalar.copy(
                        ctx["vres3"][:, kt, ec * 512 : (ec + 1) * 512], pv
                    )

        def emit_qproj(h, sc):
            pq_full = psm.tile([DH, 2 * QC], F32, tag="pp", name="pq")
            pq = pq_full[:, 0:PC]
            for i in range(NFP):
                nc.tensor.matmul(
                    pq,
                    wq3t[:, 2 * i : 2 * i + 2, h * DH : (h + 1) * DH],
                    qall3[:, 2 * i : 2 * i + 2, sc * PC : (sc + 1) * PC],
                    start=(i == 0), stop=(i == NFP - 1), perf_mode=DR,
                )
            _proj_copy(
                nc, ctx, ctx["qres3"][:, h, sc * PC : (sc + 1) * PC], pq,
                ctx["bq_sb"][:, h : h + 1],
            )

        pending_tail = [None]

        def flush_tail():
            if pending_tail[0] is not None:
                pending_tail[0]()
                pending_tail[0] = None

        # Global 1-pair software pipeline: the AV/row-sum matmuls of pair p
        # (which wait on its exp) are emitted only after the NEXT unit of
        # independent PE work (next pair's scores, a q-projection, or a KV
        # chunk), so the PE queue never stalls on a fresh exp.
        pend_B = [None]

        def pump_B():
            if pend_B[0] is not None:
                pend_B[0]()
                pend_B[0] = None

        def after_passthrough():
            pump_B()

        def emit_attn_A(inst, pi):
            h, q0, nkt = inst["h"], inst["q0"], inst["nkt"]
            kt0 = 2 * pi
            band = kt0 >= nkt - 4
            jb0 = kt0 - (nkt - 4) if band else 0
            offs = [
                (kt0 + j2 - (nkt - 4)) * DH if band else 0
                for j2 in range(2)
            ]
            pp = psm.tile([DH, 2 * QC], F32, tag="pp", name="pp")
            ppv = pp[:].rearrange("p (j q) -> p j q", j=2)
            for j2 in range(2):
                kt = kt0 + j2
                off = offs[j2]
                nc.tensor.matmul(
                    ppv[:, j2, off:QC],
                    ctx["kres3"][:, h, kt * DH : (kt + 1) * DH],
                    ctx["qres3"][:, h, q0 + off : q0 + QC],
                    start=True, stop=True,
                )
            ex = expp.tile([DH, 2 * QC], FP8, tag="ex", name="ex")
            ex3b = ex[:].rearrange("p (j q) -> p j q", j=2)
            if ctx["fast"]:
                lo = jb0 * DH
                nc.scalar.activation(
                    ex[:, lo : 2 * QC], pp[:, lo : 2 * QC],
                    ctx["EXP"], scale=EFF_SCALE,
                )
            else:
                for j2 in range(2):
                    kt = kt0 + j2
                    off = offs[j2]
                    nc.scalar.activation(
                        ex3b[:, j2, off:QC], ppv[:, j2, off:QC],
                        ctx["EXP"],
                        bias=ctx["kbias"][:, kt : kt + 1],
                        scale=EFF_SCALE,
                    )
            if band:
                # causal triangle on each diagonal 128x128 block;
                # left-of-band is never read downstream
                for j2 in range(2):
                    off = offs[j2]
                    _tri_eng(nc).tensor_mul(
                        ex3b[:, j2, off : off + DH],
                        ex3b[:, j2, off : off + DH],
                        tri,
                    )
            return dict(ex3b=ex3b, ex=ex, offs=offs, band=band, kt0=kt0)

        def emit_attn_B(inst, pi, a):
            h, nkt, npair = inst["h"], inst["nkt"], inst["npair"]
            po, psums = inst["po"], inst["psums"]
            kt0, band, offs = a["kt0"], a["band"], a["offs"]
            ex3b = a["ex3b"]
            if band:
                for j2 in range(2):
                    kt = kt0 + j2
                    off = offs[j2]
                    first = pi == 0 and j2 == 0
                    last = kt == nkt - 1
                    nc.tensor.matmul(
                        po[:, off:QC],
                        ctx["vres3"][:, kt, h * DH : (h + 1) * DH],
                        ex3b[:, j2, off:QC],
                        start=first, stop=last,
                    )
                    nc.tensor.matmul(
                        psums[:, off:QC],
                        ctx["ones16"][:, 0:1],
                        ex3b[:, j2, off:QC],
                        start=first, stop=last,
                    )
            else:
                ex3 = a["ex"][:].rearrange("p (k q) -> p k q", k=2)
                nc.tensor.matmul(
                    po[:],
                    ctx["vres3"][:, kt0 : kt0 + 2, h * DH : (h + 1) * DH],
                    ex3,
                    start=(pi == 0), stop=False,
                    perf_mode=DR,
                )
                nc.tensor.matmul(
                    psums[:],
                    ones16v[:, :, 0:1],
                    ex3,
                    start=(pi == 0), stop=False,
                    perf_mode=DR,
                )
            if pi == npair - 1:
                _complete(inst)

        def _complete(inst):
            h, q0 = inst["h"], inst["q0"]
            po, psums, oth, rsd = (inst["po"], inst["psums"], inst["oth"],
                                   inst["rsd"])
            # free psums promptly; the f32r copy feeds the tail broadcast
            sums_sb = tailsb.tile([1, QC], F32R, tag="sums", name="sums")
            nc.vector.tensor_copy(sums_sb[:], psums[:])
            flush_tail()

            def tail():
                # rank-1 broadcast of the row sums, then a full-width
                # reciprocal (same DVE cost as [1,512], all 128 lanes)
                # -> normalize + residual add -> store
                pbc = pbcp.tile([DH, QC], F32, name="pbc")
                nc.tensor.matmul(
                    pbc[:], ctx["onesr32"][:], sums_sb[:],
                    start=True, stop=True,
                )
                rec128 = bcsp.tile([DH, QC], F32, tag="bcs", name="bcs")
                nc.vector.reciprocal(rec128[:], pbc[:])
                nc.vector.tensor_mul(oth[:], po[:], rec128[:])
                _add_eng(nc).tensor_add(oth[:], oth[:], rsd[:])
                nc.sync.dma_start(
                    ctx["outT_d"][h * DH : (h + 1) * DH, q0 : q0 + QC],
                    oth[:],
                )

            pending_tail[0] = tail

        def emit_attn(h, qcI, next_qproj=None):
            q0 = qcI * QC
            nkt = 4 * (qcI + 1)
            rsd = rsdp.tile([DH, QC], ctx["BF16"], tag="rsd", name="rsd")
            nc.sync.dma_start(
                rsd[:], ctx["resid_d"][h * DH : (h + 1) * DH, q0 : q0 + QC]
            )
            inst = dict(
                h=h, q0=q0, nkt=nkt, npair=nkt // 2,
                po=pop.tile([DH, QC], F32, name="po"),
                psums=psumsp.tile([1, QC], F32, name="psums"),
                oth=othp.tile([DH, QC], F32, tag="oth", name="oth"),
                rsd=rsd,
            )
            for pi in range(inst["npair"]):
                a = emit_attn_A(inst, pi)
                pump_B()
                pend_B[0] = (lambda inst=inst, pi=pi, a=a:
                             emit_attn_B(inst, pi, a))
                if pi == 0 and next_qproj is not None:
                    emit_qproj(*next_qproj)
                    pump_B()

        kv_only = "qattn" not in phases
        attn_only = "kv" not in phases
        kv_rep = scale.get("kv", 1)
        attn_rep = scale.get("attn", 1)
        for c in range(NPC):
            if not attn_only:
                for _r in range(kv_rep):
                    emit_kv_chunk(c * PC)
                    pump_B()
            if kv_only:
                continue
            if attn_only and c == 0:
                nc.sync.dma_start(qall3, ctx["qT3"])
                nc.sync.dma_start(
                    wq_t[:].rearrange("p (f e) -> p f e", f=NF), ctx["wq3"]
                )
            for _r in range(attn_rep):
                emit_qproj(0, c)
                pump_B()
                for h in range(HPC):
                    nq = (h + 1, c) if h + 1 < HPC else None
                    emit_attn(h, c, next_qproj=nq)
        pump_B()
        flush_tail()


def _host_prep(queries, keys, Wq, bq, Wk, bk, Wv, bv, act_dt=None):
    """Build the 8 per-core input maps (host-side shard + layout prep)."""
    import ml_dtypes

    fp8 = ml_dtypes.float8_e4m3
    bf16 = ml_dtypes.bfloat16

    def to_fp8(x):
        return np.clip(x, -240.0, 240.0).astype(fp8)

    queries = np.ascontiguousarray(queries, dtype=np.float32)
    keys = np.ascontiguousarray(keys, dtype=np.float32)

    qT = np.ascontiguousarray(queries.transpose(0, 2, 1))  # [B, HID, S]
    kT = np.ascontiguousarray(keys.transpose(0, 2, 1))
    qT8 = to_fp8(qT)
    kT8 = to_fp8(kT)
    WqT = to_fp8(np.asarray(Wq, np.float32).T * WS)  # [f, e]
    WkT = to_fp8(np.asarray(Wk, np.float32).T * WS)
    WvT = to_fp8(np.asarray(Wv, np.float32).T * WS)
    bq = np.asarray(bq, np.float32) * WS
    bk = np.asarray(bk, np.float32) * WS
    bv = np.asarray(bv, np.float32) * WS

    # key padding mask -> additive bias per (b, k): 0 keep, -1e30 mask
    ksum = keys.sum(axis=-1)  # [B, S]
    kbias_all = np.where(ksum != 0.0, np.float32(0), NEG_BIAS).astype(np.float32)

    # sliding-window causal masks for the 4 diagonal-band k-tiles of a
    # q-chunk: wins[p, j*QC + q] = 1 iff q >= p + 128*j
    j_idx = np.arange(4)[None, :, None]
    wins = (
        np.arange(QC)[None, None, :] >= (np.arange(DH)[:, None, None] + DH * j_idx)
    ).astype(fp8).reshape(DH, 4 * QC)

    ones16 = np.full((DH, 32), WS, fp8)       # row-sum DR weights (value 32)
    onesr32 = np.ones((1, DH), np.float32)    # broadcast lhsT (f32r)
    onesr8 = np.ones((1, DH), fp8)            # V-bias lhsT

    in_maps = []
    for c in range(NCORES):
        b, hg = divmod(c, 2)
        e0 = hg * EH
        in_maps.append(
            {
                "qT": qT8[b],
                "kT": kT8[b],
                "wqT": np.ascontiguousarray(WqT[:, e0 : e0 + EH]),
                "wkT": np.ascontiguousarray(WkT[:, e0 : e0 + EH]),
                "wvT": np.ascontiguousarray(WvT[:, e0 : e0 + EH]),
                "bq_d": np.ascontiguousarray(bq[e0 : e0 + EH].reshape(HPC, DH).T),
                "bk_d": np.ascontiguousarray(bk[e0 : e0 + EH].reshape(HPC, DH).T),
                "bv_d": to_fp8(bv[e0 : e0 + EH].reshape(1, EH)),
                "kbias_d": np.ascontiguousarray(kbias_all[b].reshape(NKT, DH).T),
                "wins_d": wins,
                "ones16_d": ones16,
                "onesr32_d": onesr32,
                "onesr8_d": onesr8,
                "resid_d": qT[b][e0 : e0 + EH, :].astype(bf16),
            }
        )
    return in_maps


def _assemble(results):
    """results: list of 8 dicts with outT_d [EH, S] -> full [B, S, HID]."""
    out = np.empty((B, S, HID), np.float32)
    for c in range(NCORES):
        b, hg = divmod(c, 2)
        out[b, :, hg * EH : (hg + 1) * EH] = results[c]["outT_d"].T
    return out


def _flags(inputs):
    keys = np.asarray(inputs["keys"], np.float32)
    fast = not bool(np.any(keys.sum(axis=-1) == 0.0))
    zero_bias = all(
        not np.any(np.asarray(inputs[k], np.float32))
        for k in ("bq", "bk", "bv")
    )
    return fast, zero_bias


def kernel(**inputs):
    from concourse.bass_utils import run_bass_kernel_spmd

    fast, zero_bias = _flags(inputs)
    nc = _build(fast=fast, zero_bias=zero_bias)
    in_maps = _host_prep(**inputs)
    res = run_bass_kernel_spmd(nc, in_maps, core_ids=list(range(NCORES)))
    kernel.last_results = res
    return _assemble(res.results)


# revision 23
# speedup vs baseline: 5.3126x; 1.1945x over previous
"""Trainium2 Bass kernel for nn_MultiHeadAttention_88210038326473.

Reference computation (B=4, S=2048, HID=2048, H=16, DH=128):
    Q = queries @ Wq.T + bq ; K = keys @ Wk.T + bk ; V = keys @ Wv.T + bv
    per-head scores = Qh Kh^T / sqrt(HID), key-padding + causal mask,
    softmax, out = attn @ Vh, concat heads, + queries residual.

Sharding: 8 cores = 4 batches x 2 head-groups (8 heads each). Each core
computes out[b, :, hg*1024:(hg+1)*1024] (stored transposed [1024, 2048];
host transposes back and assembles).

Implementation: fp8(e4m3) operands with DoubleRow matmuls (2 contraction
subtiles of 128 per PE stream) for the three projections and the
attention AV / row-sum matmuls. Host prescales weights by 32 so fp8
weight values avoid the subnormal range; the 1/32 factors are folded
into the exp scale and the row-sum ones value. All of KT/VT/QT stay
SBUF-resident between projection and attention (no DRAM scratch).
Scores are computed transposed (sT[k,q]) in 2-bank PSUM pair tiles,
exp'd in one ScalarE call per pair (fp8 out), causal-masked with
precomputed sliding-window 0/1 masks on DVE, then consumed by
DoubleRow AV and row-sum matmuls. Normalization: DR ones-matmul row
sums -> DVE reciprocal -> PE rank-1 broadcast matmul (f32r) -> DVE
normalize + residual(bf16) add, deferred one q-chunk to keep PE fed.
Q-projection of head h+1 is woven between attention pairs of head h so
ScalarE exp time hides under PE matmuls.
"""

import math
import os as _osmod

import numpy as np

_osenv = _osmod.environ

B, S, HID, H, DH = 4, 2048, 2048, 16, 128
NCORES = 8
HPC = 8          # heads per core
EH = HPC * DH    # 1024 e-dims per core
SCALE = 1.0 / math.sqrt(HID)
WS = 32.0        # host-side weight scale (fp8 subnormal avoidance)
EFF_SCALE = float(SCALE / (WS * WS))  # exp scale: undo Q,K weight scaling
PC = 512         # projection s-chunk (matmul moving N)
NPC = S // PC    # 4
QC = 512         # attention q-chunk
NQC = S // QC    # 4
NKT = S // DH    # 16 k-tiles
NF = HID // DH   # 16 f-tiles (contraction)
NFP = NF // 2    # 8 f-pairs (DoubleRow)
NEG_BIAS = np.float32(-1.0e30)
COMPUTE_MAX_WAITS = int(_osenv.get('K_CMW', '1'))  # waits on non-CTRL instructions


CTRL_OPS = ("InstDrain", "InstNoOp", "InstEventSemaphore", "InstISA")


def _split_excess_waits(nc, max_waits=1, compute_max_waits=None):
    """walrus in this container rejects >1 sem-wait per CTRL-class instruction.
    Move excess waits onto preceding NoOps on the same engine. Compute-class
    instructions may support more waits (compute_max_waits)."""
    import concourse.mybir as mybir

    if compute_max_waits is None:
        compute_max_waits = max_waits
    n_split = 0
    for fn in nc.m.functions:
        for blk in fn.blocks:
            insts = list(blk.instructions)
            out = []
            changed = False
            for ins in insts:
                lim = (
                    max_waits
                    if type(ins).__name__ in CTRL_OPS
                    else compute_max_waits
                )
                si = ins.sync_info
                if si is not None and si.on_wait and len(si.on_wait) > lim:
                    waits = list(si.on_wait)
                    carriers, rest = waits[:-lim], waits[-lim:]
                    for i in range(0, len(carriers), max_waits):
                        chunk = carriers[i : i + max_waits]
                        out.append(
                            mybir.InstNoOp(
                                name=f"{ins.name}-ws{i}",
                                engine=ins.engine,
                                bass_nofuse=True,
                                sync_info=mybir.SyncInfo(on_wait=chunk, on_update=[]),
                            )
                        )
                        n_split += 1
                    ins.sync_info = mybir.SyncInfo(
                        on_wait=rest, on_update=list(si.on_update)
                    )
                    changed = True
                out.append(ins)
            if changed:
                blk.instructions = out
    return n_split


_CACHE = {}


def _build(fast=True, zero_bias=True, phases=("kv", "qattn"), reps=1,
           act_dt=None, scale=None):
    """Build the (core-uniform) Bass program. Returns nc.

    fast=True drops the key-padding bias from the exp (valid when no key is
    padding -- checked on host). zero_bias=True skips bias adds (all-zero
    biases, checked on host). reps/scale repeat phases for timing
    instrumentation. act_dt accepted for interface compat (ignored; fp8)."""
    scale = scale or {}
    key = ("nc", fast, zero_bias, tuple(phases), reps,
           tuple(sorted(scale.items())))
    if key in _CACHE:
        return _CACHE[key]

    import concourse.bass as bass
    import concourse.mybir as mybir
    from concourse.tile import TileContext

    F32 = mybir.dt.float32
    F32R = mybir.dt.float32r
    BF16 = mybir.dt.bfloat16
    FP8 = mybir.dt.float8e4
    EXP = mybir.ActivationFunctionType.Exp
    IDENT = mybir.ActivationFunctionType.Identity
    DR = mybir.MatmulPerfMode.DoubleRow

    nc = bass.Bass("TRN2", target_bir_lowering=False, debug=False)

    qT = nc.dram_tensor("qT", [HID, S], FP8, kind="ExternalInput")
    kT = nc.dram_tensor("kT", [HID, S], FP8, kind="ExternalInput")
    wqT = nc.dram_tensor("wqT", [HID, EH], FP8, kind="ExternalInput")
    wkT = nc.dram_tensor("wkT", [HID, EH], FP8, kind="ExternalInput")
    wvT = nc.dram_tensor("wvT", [HID, EH], FP8, kind="ExternalInput")
    bq_d = nc.dram_tensor("bq_d", [DH, HPC], F32, kind="ExternalInput")
    bk_d = nc.dram_tensor("bk_d", [DH, HPC], F32, kind="ExternalInput")
    bv_d = nc.dram_tensor("bv_d", [1, EH], FP8, kind="ExternalInput")
    kbias_d = nc.dram_tensor("kbias_d", [DH, NKT], F32, kind="ExternalInput")
    wins_d = nc.dram_tensor("wins_d", [DH, 4 * QC], FP8, kind="ExternalInput")
    ones16_d = nc.dram_tensor("ones16_d", [DH, 32], FP8, kind="ExternalInput")
    onesr32_d = nc.dram_tensor("onesr32_d", [1, DH], F32R, kind="ExternalInput")
    onesr8_d = nc.dram_tensor("onesr8_d", [1, DH], FP8, kind="ExternalInput")
    resid_d = nc.dram_tensor("resid_d", [EH, S], BF16, kind="ExternalInput")
    outT_d = nc.dram_tensor("outT_d", [EH, S], F32, kind="ExternalOutput")

    # 3D views with the 128-partition dim innermost on rows
    qT3 = qT[:].rearrange("(f p) s -> p f s", p=DH)
    kT3 = kT[:].rearrange("(f p) s -> p f s", p=DH)
    wq3 = wqT[:].rearrange("(f p) e -> p f e", p=DH)
    wk3 = wkT[:].rearrange("(f p) e -> p f e", p=DH)
    wv3 = wvT[:].rearrange("(f p) e -> p f e", p=DH)

    ctx = dict(
        F32=F32, F32R=F32R, BF16=BF16, FP8=FP8, EXP=EXP, IDENT=IDENT, DR=DR,
        fast=fast, zero_bias=zero_bias, scale=scale,
        qT3=qT3, kT3=kT3, wq3=wq3, wk3=wk3, wv3=wv3,
        resid_d=resid_d, outT_d=outT_d,
    )

    with TileContext(nc) as tc, nc.allow_low_precision(reason="fp8 attn"):
        with tc.tile_pool(name="persist", bufs=1) as persist:
            kres = persist.tile([DH, HPC * S], FP8, tag="kres")
            qres = persist.tile([DH, HPC * S], FP8, tag="qres")
            vres = persist.tile([DH, NKT * EH], FP8, tag="vres")
            wins = persist.tile([DH, 4 * QC], FP8, tag="wins")
            ones16 = persist.tile([DH, 32], FP8, tag="ones16")
            onesr32 = persist.tile([1, DH], F32R, tag="onesr32")
            onesr8 = persist.tile([1, DH], FP8, tag="onesr8")
            bq_sb = persist.tile([DH, HPC], F32, tag="bq")
            bk_sb = persist.tile([DH, HPC], F32, tag="bk")
            bv_sb = persist.tile([1, EH], FP8, tag="bv")
            kbias = persist.tile([DH, NKT], F32, tag="kbias")
            nc.sync.dma_start(wins[:], wins_d[:])
            nc.sync.dma_start(ones16[:], ones16_d[:])
            nc.sync.dma_start(onesr32[:], onesr32_d[:])
            nc.sync.dma_start(onesr8[:], onesr8_d[:])
            nc.sync.dma_start(bq_sb[:], bq_d[:])
            nc.sync.dma_start(bk_sb[:], bk_d[:])
            nc.sync.dma_start(bv_sb[:], bv_d[:])
            nc.sync.dma_start(kbias[:], kbias_d[:])

            ctx.update(
                kres3=kres[:].rearrange("p (h s) -> p h s", h=HPC),
                qres3=qres[:].rearrange("p (h s) -> p h s", h=HPC),
                vres3=vres[:].rearrange("p (kt e) -> p kt e", kt=NKT),
                wins=wins, ones16=ones16, onesr32=onesr32, onesr8=onesr8,
                bq_sb=bq_sb, bk_sb=bk_sb, bv_sb=bv_sb, kbias=kbias,
            )

            for _rep in range(reps):
                _rep_body(nc, tc, phases, ctx)

    _split_excess_waits(nc, max_waits=1, compute_max_waits=COMPUTE_MAX_WAITS)
    _CACHE[key] = nc
    return nc


import os as _os

TRI_ENGINE = _os.environ.get("K_TRI_ENGINE", "vector")
ADD_ENGINE = _os.environ.get("K_ADD_ENGINE", "vector")


def _tri_eng(nc):
    return nc.gpsimd if TRI_ENGINE == "gpsimd" else nc.vector


def _add_eng(nc):
    return nc.gpsimd if ADD_ENGINE == "gpsimd" else nc.vector


def _proj_copy(nc, ctx, dst, psrc, bias_col, engine="vector"):
    """PSUM [DH, 512] f32 -> SBUF fp8, optional per-partition bias."""
    if ctx["zero_bias"]:
        if engine == "scalar":
            nc.scalar.copy(dst, psrc)
        else:
            nc.vector.tensor_copy(dst, psrc)
    else:
        nc.scalar.activation(dst, psrc, ctx["IDENT"], bias=bias_col)


def _rep_body(nc, tc, phases, ctx):
    DR = ctx["DR"]
    F32 = ctx["F32"]
    F32R = ctx["F32R"]
    FP8 = ctx["FP8"]
    scale = ctx["scale"]

    # Wave-structured schedule: for each 512-token chunk c, project K/V for
    # that chunk, then run every head's attention q-chunk c (which only needs
    # k-tiles <= 4c+3, i.e. chunks <= c). This interleaves the ScalarE-heavy
    # exp work with the PE-heavy projections across the whole timeline.
    # PSUM (8 banks): pp-slots 2x2 (scores pairs + K/V/Q proj tiles + the
    # tail broadcast), po 2, psums 1, pbc 1.
    with tc.tile_pool(name="wk", bufs=1) as wkp, \
         tc.tile_pool(name="wv", bufs=1) as wvp, \
         tc.tile_pool(name="wq", bufs=1) as wqp, \
         tc.tile_pool(name="qall", bufs=1) as qallp, \
         tc.tile_pool(name="kc", bufs=3) as kcp, \
         tc.tile_pool(name="ex", bufs=4) as expp, \
         tc.tile_pool(name="tailsb", bufs=3) as tailsb, \
         tc.tile_pool(name="bcs", bufs=2) as bcsp, \
         tc.tile_pool(name="rsd", bufs=3) as rsdp, \
         tc.tile_pool(name="oth", bufs=3) as othp, \
         tc.tile_pool(name="psm", bufs=2, space="PSUM") as psm, \
         tc.tile_pool(name="po", bufs=2, space="PSUM") as pop, \
         tc.tile_pool(name="psums", bufs=1, space="PSUM") as psumsp, \
         tc.tile_pool(name="pbc", bufs=1, space="PSUM") as pbcp:
        wk_t = wkp.tile([DH, NF * EH], FP8, tag="wk", name="wk")
        wk3t_w = wk_t[:].rearrange("p (f e) -> p f e", f=NF)
        # first e-half of Wk only: the first K matmuls (et 0-3) start after
        # 2 MB of critical DMA instead of 5 MB
        nc.sync.dma_start(wk3t_w[:, :, 0 : EH // 2], ctx["wk3"][:, :, 0 : EH // 2])
        wv_t = wvp.tile([DH, NF * EH], FP8, tag="wv", name="wv")
        wq_t = wqp.tile([DH, NF * EH], FP8, tag="wq", name="wq")
        qall = qallp.tile([DH, NF * S], FP8, tag="qall", name="qall")
        qall3 = qall[:].rearrange("p (f s) -> p f s", f=NF)
        wk3t = wk_t[:].rearrange("p (f e) -> p f e", f=NF)
        wv3t = wv_t[:].rearrange("p (f e) -> p f e", f=NF)
        wq3t = wq_t[:].rearrange("p (f e) -> p f e", f=NF)
        ones16v = ctx["ones16"][:].rearrange("p (k m) -> p k m", k=2)
        tri = ctx["wins"][:, 0:DH]

        # zero both score-PSUM slots once: band pairs exp() regions their
        # matmuls never wrote; stale PSUM must stay finite
        for _z in range(2):
            ppz = psm.tile([DH, 2 * QC], F32, tag="pp", name="ppz")
            nc.vector.memset(ppz[:], 0.0)

        def emit_kv_chunk(s0):
            kc = kcp.tile([DH, NF * PC], FP8, tag="kc", name="kc")
            kc3 = kc[:].rearrange("p (f s) -> p f s", f=NF)
            nc.sync.dma_start(kc3, ctx["kT3"][:, :, s0 : s0 + PC])
            if s0 == 0:
                # the rest of the weights + q-side loads ride behind the
                # wave-0 K-critical loads
                nc.sync.dma_start(
                    wk3t_w[:, :, EH // 2 : EH], ctx["wk3"][:, :, EH // 2 : EH]
                )
                nc.sync.dma_start(
                    wv_t[:].rearrange("p (f e) -> p f e", f=NF), ctx["wv3"]
                )
                nc.sync.dma_start(qall3[:, :, 0:PC], ctx["qT3"][:, :, 0:PC])
                nc.sync.dma_start(
                    wq_t[:].rearrange("p (f e) -> p f e", f=NF), ctx["wq3"]
                )
            else:
                nc.sync.dma_start(
                    qall3[:, :, s0 : s0 + PC], ctx["qT3"][:, :, s0 : s0 + PC]
                )
            for et in range(HPC):
                pk_full = psm.tile([DH, 2 * QC], F32, tag="pp", name="pk")
                pk = pk_full[:, 0:PC]
                for i in range(NFP):
                    nc.tensor.matmul(
                        pk,
                        wk3t[:, 2 * i : 2 * i + 2, et * DH : (et + 1) * DH],
                        kc3[:, 2 * i : 2 * i + 2, :],
                        start=(i == 0), stop=(i == NFP - 1), perf_mode=DR,
                    )
                _proj_copy(
                    nc, ctx, ctx["kres3"][:, et, s0 : s0 + PC], pk,
                    ctx["bk_sb"][:, et : et + 1], engine="scalar",
                )
            for sti in range(PC // DH):
                kt = s0 // DH + sti
                for ec in range(2):
                    pv_full = psm.tile([DH, 2 * QC], F32, tag="pp", name="pv")
                    pv = pv_full[:, 0:512]
                    for i in range(NFP):
                        nc.tensor.matmul(
                            pv,
                            kc3[:, 2 * i : 2 * i + 2,
                                sti * DH : (sti + 1) * DH],
                            wv3t[:, 2 * i : 2 * i + 2,
                                 ec * 512 : (ec + 1) * 512],
                            start=(i == 0),
                            stop=(ctx["zero_bias"] and i == NFP - 1),
                            perf_mode=DR,
                        )
                    if not ctx["zero_bias"]:
                        nc.tensor.matmul(
                            pv,
                            ctx["onesr8"][:],
                            ctx["bv_sb"][:, ec * 512 : (ec + 1) * 512],
                            start=False, stop=True,
                        )
                    nc.scalar.copy(
                        ctx["vres3"][:, kt, ec * 512 : (ec + 1) * 512], pv
                    )

        def emit_qproj(h, sc):
            pq_full = psm.tile([DH, 2 * QC], F32, tag="pp", name="pq")
            pq = pq_full[:, 0:PC]
            for i in range(NFP):
                nc.tensor.matmul(
                    pq,
                    wq3t[:, 2 * i : 2 * i + 2, h * DH : (h + 1) * DH],
                    qall3[:, 2 * i : 2 * i + 2, sc * PC : (sc + 1) * PC],
                    start=(i == 0), stop=(i == NFP - 1), perf_mode=DR,
                )
            _proj_copy(
                nc, ctx, ctx["qres3"][:, h, sc * PC : (sc + 1) * PC], pq,
                ctx["bq_sb"][:, h : h + 1],
            )

        pending_tail = [None]

        def flush_tail():
            if pending_tail[0] is not None:
                pending_tail[0]()
                pending_tail[0] = None

        # Global 1-pair software pipeline: the AV/row-sum matmuls of pair p
        # (which wait on its exp) are emitted only after the NEXT unit of
        # independent PE work (next pair's scores, a q-projection, or a KV
        # chunk), so the PE queue never stalls on a fresh exp.
        pend_B = [None]

        def pump_B():
            if pend_B[0] is not None:
                pend_B[0]()
                pend_B[0] = None

        def after_passthrough():
            pump_B()

        def emit_attn_A(inst, pi):
            h, q0, nkt = inst["h"], inst["q0"], inst["nkt"]
            kt0 = 2 * pi
            band = kt0 >= nkt - 4
            jb0 = kt0 - (nkt - 4) if band else 0
            offs = [
                (kt0 + j2 - (nkt - 4)) * DH if band else 0
                for j2 in range(2)
            ]
            pp = psm.tile([DH, 2 * QC], F32, tag="pp", name="pp")
            ppv = pp[:].rearrange("p (j q) -> p j q", j=2)
            for j2 in range(2):
                kt = kt0 + j2
                off = offs[j2]
                nc.tensor.matmul(
                    ppv[:, j2, off:QC],
                    ctx["kres3"][:, h, kt * DH : (kt + 1) * DH],
                    ctx["qres3"][:, h, q0 + off : q0 + QC],
                    start=True, stop=True,
                )
            ex = expp.tile([DH, 2 * QC], FP8, tag="ex", name="ex")
            ex3b = ex[:].rearrange("p (j q) -> p j q", j=2)
            if ctx["fast"]:
                lo = jb0 * DH
                nc.scalar.activation(
                    ex[:, lo : 2 * QC], pp[:, lo : 2 * QC],
                    ctx["EXP"], scale=EFF_SCALE,
                )
            else:
                for j2 in range(2):
                    kt = kt0 + j2
                    off = offs[j2]
                    nc.scalar.activation(
                        ex3b[:, j2, off:QC], ppv[:, j2, off:QC],
                        ctx["EXP"],
                        bias=ctx["kbias"][:, kt : kt + 1],
                        scale=EFF_SCALE,
                    )
            if band:
                # causal triangle on each diagonal 128x128 block;
                # left-of-band is never read downstream
                for j2 in range(2):
                    off = offs[j2]
                    _tri_eng(nc).tensor_mul(
                        ex3b[:, j2, off : off + DH],
                        ex3b[:, j2, off : off + DH],
                        tri,
                    )
            return dict(ex3b=ex3b, ex=ex, offs=offs, band=band, kt0=kt0)

        def emit_attn_B(inst, pi, a):
            h, nkt, npair = inst["h"], inst["nkt"], inst["npair"]
            po, psums = inst["po"], inst["psums"]
            kt0, band, offs = a["kt0"], a["band"], a["offs"]
            ex3b = a["ex3b"]
            if band:
                for j2 in range(2):
                    kt = kt0 + j2
                    off = offs[j2]
                    first = pi == 0 and j2 == 0
                    last = kt == nkt - 1
                    nc.tensor.matmul(
                        po[:, off:QC],
                        ctx["vres3"][:, kt, h * DH : (h + 1) * DH],
                        ex3b[:, j2, off:QC],
                        start=first, stop=last,
                    )
                    nc.tensor.matmul(
                        psums[:, off:QC],
                        ctx["ones16"][:, 0:1],
                        ex3b[:, j2, off:QC],
                        start=first, stop=last,
                    )
            else:
                ex3 = a["ex"][:].rearrange("p (k q) -> p k q", k=2)
                nc.tensor.matmul(
                    po[:],
                    ctx["vres3"][:, kt0 : kt0 + 2, h * DH : (h + 1) * DH],
                    ex3,
                    start=(pi == 0), stop=False,
                    perf_mode=DR,
                )
                nc.tensor.matmul(
                    psums[:],
                    ones16v[:, :, 0:1],
                    ex3,
                    start=(pi == 0), stop=False,
                    perf_mode=DR,
                )
            if pi == npair - 1:
                _complete(inst)

        def _complete(inst):
            h, q0 = inst["h"], inst["q0"]
            po, psums, oth, rsd = (inst["po"], inst["psums"], inst["oth"],
                                   inst["rsd"])
            # free psums promptly; the f32r copy feeds the tail broadcast
            sums_sb = tailsb.tile([1, QC], F32R, tag="sums", name="sums")
            nc.vector.tensor_copy(sums_sb[:], psums[:])
            flush_tail()

            def tail():
                # rank-1 broadcast of the row sums, then a full-width
                # reciprocal (same DVE cost as [1,512], all 128 lanes)
                # -> normalize + residual add -> store
                pbc = pbcp.tile([DH, QC], F32, name="pbc")
                nc.tensor.matmul(
                    pbc[:], ctx["onesr32"][:], sums_sb[:],
                    start=True, stop=True,
                )
                rec128 = bcsp.tile([DH, QC], F32, tag="bcs", name="bcs")
                nc.vector.reciprocal(rec128[:], pbc[:])
                nc.vector.tensor_mul(oth[:], po[:], rec128[:])
                _add_eng(nc).tensor_add(oth[:], oth[:], rsd[:])
                nc.sync.dma_start(
                    ctx["outT_d"][h * DH : (h + 1) * DH, q0 : q0 + QC],
                    oth[:],
                )

            pending_tail[0] = tail

        def emit_attn(h, qcI, next_qproj=None):
            q0 = qcI * QC
            nkt = 4 * (qcI + 1)
            rsd = rsdp.tile([DH, QC], ctx["BF16"], tag="rsd", name="rsd")
            nc.sync.dma_start(
                rsd[:], ctx["resid_d"][h * DH : (h + 1) * DH, q0 : q0 + QC]
            )
            inst = dict(
                h=h, q0=q0, nkt=nkt, npair=nkt // 2,
                po=pop.tile([DH, QC], F32, name="po"),
                psums=psumsp.tile([1, QC], F32, name="psums"),
                oth=othp.tile([DH, QC], F32, tag="oth", name="oth"),
                rsd=rsd,
            )
            for pi in range(inst["npair"]):
                a = emit_attn_A(inst, pi)
                pump_B()
                pend_B[0] = (lambda inst=inst, pi=pi, a=a:
                             emit_attn_B(inst, pi, a))
                if pi == 0 and next_qproj is not None:
                    emit_qproj(*next_qproj)
                    pump_B()

        kv_only = "qattn" not in phases
        attn_only = "kv" not in phases
        kv_rep = scale.get("kv", 1)
        attn_rep = scale.get("attn", 1)
        for c in range(NPC):
            if not attn_only:
                for _r in range(kv_rep):
                    emit_kv_chunk(c * PC)
                    pump_B()
            if kv_only:
                continue
            if attn_only and c == 0:
                nc.sync.dma_start(qall3, ctx["qT3"])
                nc.sync.dma_start(
                    wq_t[:].rearrange("p (f e) -> p f e", f=NF), ctx["wq3"]
                )
            for _r in range(attn_rep):
                emit_qproj(0, c)
                pump_B()
                for h in range(HPC):
                    nq = (h + 1, c) if h + 1 < HPC else None
                    emit_attn(h, c, next_qproj=nq)
        pump_B()
        flush_tail()


def _host_prep(queries, keys, Wq, bq, Wk, bk, Wv, bv, act_dt=None):
    """Build the 8 per-core input maps (host-side shard + layout prep)."""
    import ml_dtypes

    fp8 = ml_dtypes.float8_e4m3
    bf16 = ml_dtypes.bfloat16

    def to_fp8(x):
        return np.clip(x, -240.0, 240.0).astype(fp8)

    queries = np.ascontiguousarray(queries, dtype=np.float32)
    keys = np.ascontiguousarray(keys, dtype=np.float32)

    qT = np.ascontiguousarray(queries.transpose(0, 2, 1))  # [B, HID, S]
    kT = np.ascontiguousarray(keys.transpose(0, 2, 1))
    qT8 = to_fp8(qT)
    kT8 = to_fp8(kT)
    WqT = to_fp8(np.asarray(Wq, np.float32).T * WS)  # [f, e]
    WkT = to_fp8(np.asarray(Wk, np.float32).T * WS)
    WvT = to_fp8(np.asarray(Wv, np.float32).T * WS)
    bq = np.asarray(bq, np.float32) * WS
    bk = np.asarray(bk, np.float32) * WS
    bv = np.asarray(bv, np.float32) * WS

    # key padding mask -> additive bias per (b, k): 0 keep, -1e30 mask
    ksum = keys.sum(axis=-1)  # [B, S]
    kbias_all = np.where(ksum != 0.0, np.float32(0), NEG_BIAS).astype(np.float32)

    # sliding-window causal masks for the 4 diagonal-band k-tiles of a
    # q-chunk: wins[p, j*QC + q] = 1 iff q >= p + 128*j
    j_idx = np.arange(4)[None, :, None]
    wins = (
        np.arange(QC)[None, None, :] >= (np.arange(DH)[:, None, None] + DH * j_idx)
    ).astype(fp8).reshape(DH, 4 * QC)

    ones16 = np.full((DH, 32), WS, fp8)       # row-sum DR weights (value 32)
    onesr32 = np.ones((1, DH), np.float32)    # broadcast lhsT (f32r)
    onesr8 = np.ones((1, DH), fp8)            # V-bias lhsT

    in_maps = []
    for c in range(NCORES):
        b, hg = divmod(c, 2)
        e0 = hg * EH
        in_maps.append(
            {
                "qT": qT8[b],
                "kT": kT8[b],
                "wqT": np.ascontiguousarray(WqT[:, e0 : e0 + EH]),
                "wkT": np.ascontiguousarray(WkT[:, e0 : e0 + EH]),
                "wvT": np.ascontiguousarray(WvT[:, e0 : e0 + EH]),
                "bq_d": np.ascontiguousarray(bq[e0 : e0 + EH].reshape(HPC, DH).T),
                "bk_d": np.ascontiguousarray(bk[e0 : e0 + EH].reshape(HPC, DH).T),
                "bv_d": to_fp8(bv[e0 : e0 + EH].reshape(1, EH)),
                "kbias_d": np.ascontiguousarray(kbias_all[b].reshape(NKT, DH).T),
                "wins_d": wins,
                "ones16_d": ones16,
                "onesr32_d": onesr32,
                "onesr8_d": onesr8,
                "resid_d": qT[b][e0 : e0 + EH, :].astype(bf16),
            }
        )
    return in_maps


def _assemble(results):
    """results: list of 8 dicts with outT_d [EH, S] -> full [B, S, HID]."""
    out = np.empty((B, S, HID), np.float32)
    for c in range(NCORES):
        b, hg = divmod(c, 2)
        out[b, :, hg * EH : (hg + 1) * EH] = results[c]["outT_d"].T
    return out


def _flags(inputs):
    keys = np.asarray(inputs["keys"], np.float32)
    fast = not bool(np.any(keys.sum(axis=-1) == 0.0))
    zero_bias = all(
        not np.any(np.asarray(inputs[k], np.float32))
        for k in ("bq", "bk", "bv")
    )
    return fast, zero_bias


def kernel(**inputs):
    from concourse.bass_utils import run_bass_kernel_spmd

    fast, zero_bias = _flags(inputs)
    nc = _build(fast=fast, zero_bias=zero_bias)
    in_maps = _host_prep(**inputs)
    res = run_bass_kernel_spmd(nc, in_maps, core_ids=list(range(NCORES)))
    kernel.last_results = res
    return _assemble(res.results)
